# revision 46
# baseline (speedup 1.0000x reference)
"""Trainium2 Bass kernel for nn_BitLinear (LayerNorm -> 1.58-bit BitLinear).

Math notes
----------
Reference computes, per the module:
    xn    = LN(x) * ln_gamma + ln_beta            (eps = 1e-3)
    beta  = mean(|W|);  w_q = clip(round(W / (beta + 1e-5)), -1, 1)
    gamma = max(|xn|)   (global absmax)
    xq    = clip(xn * 128 / gamma, -128 + 1e-5, 128 - 1e-5)
    y     = (xq @ w_q) * (gamma * beta / 128)

The gamma factor cancels exactly: (xn*128/gamma) @ w_q * (gamma*beta/128)
== (xn @ w_q) * beta.  The clip only affects elements within relative
7.8e-8 of the global absmax, changing them by ~1e-7 relative -- far below
f32 matmul roundoff.  So the kernel computes y = (LN(x) @ w_q) * beta,
which is fully data-parallel over tokens (no collectives needed).

w_q is ternary: w_q = sign(W) * 1[|W| > c] with c = 0.5*(beta+1e-5).
The kernel stores wq' = 0.5*w_q via ONE fused DVE op per k-group:
    wq = (|W| is_gt c) * sgnh,   sgnh = (W>=0)-0.5 in {-.5,+.5}
(scalar_tensor_tensor; the 2x is folded into the output scale 2*beta).
sgnh and |W| (f32, row-sums accumulated for beta) are computed while W
streams in.  All compares are f32: a bf16 compare would misclassify
~300 weights near the threshold (~2e-2 output error, at the budget).

LN normalization scale is folded into the epilogue: xn = (x - mu) in
bf16 (scale-invariant relative precision; the matmul is linear), and
esc[t] = rsqrt(var+eps)[t] * 2*beta scales each output row (ACT
scalar.mul with a [P,1] operand).

Sharding: data-parallel over the 32768 tokens, 4096 per core; weight
replicated (each core redundantly computes beta/w_q from the full W --
cheaper than a collective).

Schedule (measured: DMA starts ~8-10us after kernel start due to the
fixed engine preamble; the two HWDGE rings sustain ~410 GB/s combined
but share it unevenly; DVE f32 passes are ~0.7us, fused stt ~1.2us/k;
GPSIMD tensor ops are 2.5us AND stall concurrent DVE ops via SBUF
contention -- never used):
  * Ring q1 (sync/SP):    x0, W3a, W0, W1, W2, x1, x2, ...
    Ring q10 (scalar/ACT): W3b, W4..W7, y0, y1, ...
    (W3 split across rings balances their drain at ~23.5us; x supers
    queue behind W by ring FIFO so they cannot steal prologue
    bandwidth.)
  * W prep per chunk, in landing order: DVE extracts sgnh, then |W| in
    place via one fused pass for early q1 chunks, ACT Abs+accum for q10
    chunks; the LAST chunk on each ring (2 and 7) writes |W| to scratch
    instead (not in place) so its sgnh runs concurrently and the
    beta -> c chain closes ~1us after the last W byte.
  * The PE would idle >12us waiting for W; idle >3.4us drops its clock
    to 1.2 GHz (HAM gate).  Dummy identity transposes keep it busy+warm
    until real work arrives.
  * Ternarize emits as k-grouped fused ops with super-1 stats
    interleaved; the first super's matmuls interleave its two tiles
    k-by-k so consumption (~1.7us/k) matches wq production (~1.2us/k)
    with no PE stall.
  * Steady loop per tile: M(j)i then T(j+1)i -- the xT PSUM->SBUF copy
    for super j+1 runs on DVE during back(j)'s matmuls, so the PE never
    waits on copies.
  * Final super runs h-major with per-half drains on both rings, so the
    post-matmul tail is one 256 KiB transfer deep per ring.
"""

import numpy as np

B, S, D, U = 4, 8192, 1024, 1024
N_CORES = 8
TOK = (B * S) // N_CORES  # 4096 tokens per core
P = 128
KB = D // P               # 8 contraction blocks
NTILES = TOK // P         # 32 token tiles per core
SUPER = 2                 # token tiles per DMA transfer (1 MiB chunks)
NJ = NTILES // SUPER      # 16 super-tiles
N_DUMMY1 = 260            # PE warmup transposes before T0
N_DUMMY2 = 190            # ... between T0 and the ones-matmul
LN_EPS = 1e-3
EPS = 1e-5

_NC_CACHE = {}


def _build(apply_gamma: bool, apply_beta: bool):
    """Build the single-core Bass program (SPMD: same NEFF on all 8 cores)."""
    import concourse.bacc as bacc
    import concourse.bass_isa as bass_isa
    import concourse.mybir as mybir
    import concourse.tile as tile
    from concourse.bass import ts
    from concourse.masks import make_identity

    fp32 = mybir.dt.float32
    bf16 = mybir.dt.bfloat16
    AF = mybir.ActivationFunctionType
    OP = mybir.AluOpType

    nc = bacc.Bacc()
    x_h = nc.dram_tensor("x", [TOK, D], fp32, kind="ExternalInput")
    w_h = nc.dram_tensor("weight", [D, U], fp32, kind="ExternalInput")
    g_h = (
        nc.dram_tensor("ln_gamma", [D], fp32, kind="ExternalInput")
        if apply_gamma
        else None
    )
    lb_h = (
        nc.dram_tensor("ln_beta", [D], fp32, kind="ExternalInput")
        if apply_beta
        else None
    )
    y_h = nc.dram_tensor("y", [TOK, U], fp32, kind="ExternalOutput")

    DVE_ABS = (0, 1, 3, 4)   # chunks whose fused |W|+accum rides DVE
    SCRATCH_ABS = (2, 5, 6, 7)  # ACT abs to scratch (no sgnh ordering dep)

    with tile.TileContext(nc) as tc:
        with (
            tc.tile_pool(name="singles", bufs=1) as singles,
            tc.tile_pool(name="xin", bufs=4) as xin_pool,
            tc.tile_pool(name="xn", bufs=6) as xn_pool,
            tc.tile_pool(name="xt", bufs=8) as xt_pool,
            tc.tile_pool(name="yout", bufs=3) as y_pool,
            tc.tile_pool(name="stats", bufs=6) as stats_pool,
            tc.tile_pool(name="ps_t", bufs=4, space="PSUM") as ps_t_pool,
            tc.tile_pool(name="ps_y", bufs=2, space="PSUM") as ps_y_pool,
        ):
            # ---- constants ----
            ident = singles.tile([P, P], bf16)
            make_identity(nc, ident)
            eps_t = singles.tile([P, 1], fp32)
            nc.vector.memset(eps_t, LN_EPS)
            ones_f32 = singles.tile([P, P], fp32)
            nc.vector.memset(ones_f32, 1.0)

            # ---- DMA issue order defines ring FIFO order ----
            w_view = w_h[:, :].rearrange("(ko ki) u -> ki ko u", ki=P)
            x_view = x_h[:, :].rearrange("(o p) d -> p o d", p=P)
            y_view = y_h[:, :].rearrange("(o p) u -> p o u", p=P)

            def issue_x(j, eng):
                x_sb = xin_pool.tile([P, SUPER, D], fp32, name="x_sb")
                eng.dma_start(
                    out=x_sb, in_=x_view[:, j * SUPER : (j + 1) * SUPER, :]
                )
                return x_sb

            w_sb = singles.tile([P, KB, U], fp32)
            x_supers = {0: issue_x(0, nc.sync)}
            # q1: x0, W3a, W0, W1, W2, x1 ...   q10: W3b, W4..W7, y ...
            nc.sync.dma_start(out=w_sb[:, 3, 0:512], in_=w_view[:, 3, 0:512])
            nc.scalar.dma_start(
                out=w_sb[:, 3, 512:1024], in_=w_view[:, 3, 512:1024]
            )
            for k in (0, 1, 2):
                nc.sync.dma_start(out=w_sb[:, k, :], in_=w_view[:, k, :])
            for k in (4, 5, 6, 7):
                nc.scalar.dma_start(out=w_sb[:, k, :], in_=w_view[:, k, :])
            x_supers[1] = issue_x(1, nc.sync)

            if apply_gamma:
                g_sb = singles.tile([P, KB], fp32)
                nc.scalar.dma_start(
                    out=g_sb, in_=g_h[:].rearrange("(ko ki) -> ki ko", ki=P)
                )
            if apply_beta:
                lb_f32 = singles.tile([P, KB], fp32)
                nc.scalar.dma_start(
                    out=lb_f32, in_=lb_h[:].rearrange("(ko ki) -> ki ko", ki=P)
                )
                lb_sb = singles.tile([P, KB], bf16)
                nc.vector.tensor_copy(out=lb_sb, in_=lb_f32)

            # ---- W prep: sgnh (sign), then |W| + row-sums for beta ----
            sgnh = singles.tile([P, KB, U], bf16)
            asum = singles.tile([P, KB], fp32)
            scratch = singles.tile([P, 4, U], fp32)  # |W| for ACT chunks
            abs_src = {}  # k -> AP holding |W| for the ternarize compare

            def emit_sgnh(k):
                # (W>=0)-0.5 in {-.5,+.5}, exact in bf16
                nc.vector.tensor_scalar(
                    out=sgnh[:, k, :], in0=w_sb[:, k, :], scalar1=0.0,
                    scalar2=0.5, op0=OP.is_ge, op1=OP.subtract,
                )
                if apply_gamma and not apply_beta:
                    # fold ln_gamma rows in (the beff path needs raw w_q, so
                    # the combined variant applies gamma later instead)
                    nc.vector.tensor_scalar(
                        out=sgnh[:, k, :], in0=sgnh[:, k, :],
                        scalar1=g_sb[:, k : k + 1], scalar2=None, op0=OP.mult,
                    )

            def emit_abs(k):
                if k in SCRATCH_ABS:
                    # NOT in place: sgnh (DVE) runs concurrently with this
                    # ACT pass -- critical for the last chunk on each ring
                    i = SCRATCH_ABS.index(k)
                    nc.scalar.activation(
                        out=scratch[:, i, :], in_=w_sb[:, k, :], func=AF.Abs,
                        accum_out=asum[:, k : k + 1],
                    )
                    abs_src[k] = scratch[:, i : i + 1, :]
                elif k in DVE_ABS:
                    # |W| = (2W) * sgnh exactly, row-sum accumulated: one
                    # fused DVE pass, in place
                    nc.vector.scalar_tensor_tensor(
                        out=w_sb[:, k, :], in0=w_sb[:, k, :], scalar=2.0,
                        in1=sgnh[:, k, :], op0=OP.mult, op1=OP.mult,
                        accum_out=asum[:, k : k + 1],
                    )
                    abs_src[k] = w_sb[:, k : k + 1, :]
                else:
                    nc.scalar.activation(
                        out=w_sb[:, k, :], in_=w_sb[:, k, :], func=AF.Abs,
                        accum_out=asum[:, k : k + 1],
                    )
                    abs_src[k] = w_sb[:, k : k + 1, :]

            # ---- LN stats on DVE; the normalize pass rides ACT ----
            def front_stats(x_sb, i):
                xt_ = x_sb[:, i, :]
                st = stats_pool.tile([P, 2, 6], fp32, tag="st")
                xr = xt_.rearrange("p (n f) -> p n f", f=512)
                nc.vector.bn_stats(out=st[:, 0, :], in_=xr[:, 0, :])
                nc.vector.bn_stats(out=st[:, 1, :], in_=xr[:, 1, :])
                mv = stats_pool.tile([P, 2], fp32, tag="mv")
                nc.vector.bn_aggr(out=mv, in_=st)
                nb = stats_pool.tile([P, 1], fp32, tag="nb")
                nc.vector.tensor_scalar(
                    out=nb, in0=mv[:, 0:1], scalar1=-1.0, scalar2=None,
                    op0=OP.mult,
                )
                # xn = x - mu (bf16); rsqrt scale folds into the epilogue
                xn = xn_pool.tile([P, D], bf16)
                nc.scalar.activation(
                    out=xn, in_=xt_, func=AF.Identity, bias=nb, scale=1.0
                )
                # sq = sqrt(var + eps) (tiny, ACT)
                sq = stats_pool.tile([P, 1], fp32, tag="sq")
                nc.scalar.activation(
                    out=sq, in_=mv[:, 1:2], func=AF.Sqrt, bias=eps_t, scale=1.0
                )
                return xn, sq

            # tiny ACT table warm-ups: they run in ACT's natural idle window
            # (after descriptor-gen, before the first chunk lands) so the
            # Abs/Sqrt/Copy tables aren't loaded on the critical path later
            warm = singles.tile([P, 3], fp32)
            nc.scalar.activation(out=warm[:, 0:1], in_=eps_t, func=AF.Abs)
            nc.scalar.activation(
                out=warm[:, 1:2], in_=eps_t, func=AF.Sqrt, bias=eps_t, scale=1.0
            )
            nc.scalar.copy(out=warm[:, 2:3], in_=eps_t)

            # W prep in expected landing order (q10: W3b@12, W4@15, W5@19,
            # W6@22, W7@23.5; q1: x0@14, W3a@16, W0@19, W1@21, W2@23.5).
            # ACT chunks write |W| to scratch so they have no sgnh WAR dep;
            # DVE chunks fuse sgnh-mult+accum in one in-place pass.
            emit_sgnh(4)
            emit_abs(4)
            frs = {0: [front_stats(x_supers[0], i) for i in range(SUPER)]}
            emit_sgnh(3)
            emit_abs(3)
            emit_abs(5)
            emit_sgnh(0)
            emit_abs(0)
            emit_abs(6)
            emit_sgnh(1)
            emit_abs(1)

            # ---- PE warmup dummies (keep the HAM clock at 2.4 GHz) ----
            ps_dummy = ps_t_pool.tile([P, KB, P], bf16, tag="ps_t", name="ps_d")
            for i in range(N_DUMMY1):
                nc.tensor.transpose(ps_dummy[:, i % KB, :], ident, ident)

            # ---- transposes + copies ----
            def transpose_tile(fr, copy_eng=None):
                xn, sq = fr
                ps_xt = ps_t_pool.tile([P, KB, P], bf16, tag="ps_t")
                for k in range(KB):
                    nc.tensor.transpose(ps_xt[:, k, :], xn[:, ts(k, P)], ident)
                xT = xt_pool.tile([P, KB, P], bf16)
                (copy_eng or nc.vector.tensor_copy)(out=xT, in_=ps_xt)
                return (xT, sq)

            def transpose_tile_dma(fr, eng):
                # steady state: the DMA xbar does the [tok,d]->[d,tok]
                # transpose (out[di,k,t] = in[t, k*128+di]), freeing the PE
                # of ~14us of transposes and the DVE of the PSUM copies
                xn, sq = fr
                xT = xt_pool.tile([P, KB, P], bf16)
                eng.dma_start_transpose(xT[:, :, :], xn[:, :])
                return (xT, sq)

            fronts = {0: [transpose_tile(fr) for fr in frs.pop(0)]}

            # W prep tail: the last-landing chunks (scratch-abs on ACT runs
            # concurrently with their sgnh on DVE)
            emit_abs(2)
            emit_abs(7)
            emit_sgnh(5)
            emit_sgnh(6)
            emit_sgnh(2)
            emit_sgnh(7)
            asum1 = singles.tile([P, 1], fp32)
            nc.vector.tensor_reduce(
                out=asum1, in_=asum, axis=mybir.AxisListType.X, op=OP.add
            )

            for i in range(N_DUMMY2):
                nc.tensor.transpose(ps_dummy[:, i % KB, :], ident, ident)

            # cross-partition total broadcast to all partitions in ONE matmul
            ps_tot = ps_y_pool.tile([P, U], fp32, tag="ps_y", name="ps_tot")
            nc.tensor.matmul(
                ps_tot[:, 0:1], lhsT=ones_f32, rhs=asum1, start=True, stop=True
            )
            t128 = singles.tile([P, 1], fp32)
            nc.vector.tensor_copy(out=t128, in_=ps_tot[:, 0:1])
            # c = (beta+EPS)/2 ;  output scale 2*beta (wq holds 0.5*w_q)
            c128 = singles.tile([P, 1], fp32)
            nc.vector.tensor_scalar(
                out=c128, in0=t128, scalar1=0.5 / (D * U), scalar2=0.5 * EPS,
                op0=OP.mult, op1=OP.add,
            )
            bh128 = singles.tile([P, 1], fp32)
            nc.vector.tensor_scalar(
                out=bh128, in0=t128, scalar1=2.0 / (D * U), scalar2=None,
                op0=OP.mult,
            )

            # ---- ternarize: fused (|W| is_gt c) * sgnh per k-group ----
            wq = singles.tile([P, KB, U], bf16)  # holds 0.5*w_q (*gamma)

            def emit_tern(k0, klen):
                if k0 in SCRATCH_ABS:
                    i0 = SCRATCH_ABS.index(k0)
                    assert all(
                        SCRATCH_ABS.index(k0 + q) == i0 + q for q in range(klen)
                    )
                    src = scratch[:, i0 : i0 + klen, :]
                else:
                    assert all(k0 + q in DVE_ABS for q in range(klen))
                    src = w_sb[:, k0 : k0 + klen, :]
                nc.vector.scalar_tensor_tensor(
                    out=wq[:, k0 : k0 + klen, :], in0=src, scalar=c128,
                    in1=sgnh[:, k0 : k0 + klen, :], op0=OP.is_gt, op1=OP.mult,
                )

            beff128 = None

            # ---- back side ----
            def esc_for(sq):
                # esc = rsqrt(var+eps) * 2*beta, per token (tiny DVE chain)
                esc = stats_pool.tile([P, 1], fp32, tag="esc")
                nc.vector.reciprocal(esc, sq)
                nc.vector.tensor_scalar(
                    out=esc, in0=esc, scalar1=bh128, scalar2=None, op0=OP.mult
                )
                return esc

            def epilogue(y_sb, i, j, ps_y, esc, h=None):
                sl = slice(None) if h is None else ts(h, 512)
                nc.scalar.mul(out=y_sb[:, i, sl], in_=ps_y[:, sl], mul=esc)
                if apply_beta:
                    nc.vector.tensor_tensor(
                        y_sb[:, i, sl], y_sb[:, i, sl], beff128[:, sl], OP.add
                    )

            def back_tile(xt_sq, y_sb, i, j):
                xT, sq = xt_sq
                last = j == NJ - 1
                ps_y = ps_y_pool.tile([P, U], fp32, tag="ps_y")
                esc = esc_for(sq)
                if last:
                    # h-major: each half's epilogue + 256 KiB drain starts as
                    # soon as that half's accumulation closes (short tail)
                    for h in range(2):
                        for k in range(KB):
                            nc.tensor.matmul(
                                ps_y[:, ts(h, 512)],
                                lhsT=xT[:, k, :],
                                rhs=wq[:, k, ts(h, 512)],
                                start=(k == 0),
                                stop=(k == KB - 1),
                            )
                        epilogue(y_sb, i, j, ps_y, esc, h)
                        eng = nc.scalar if h == 0 else nc.sync
                        eng.dma_start(
                            out=y_view[:, j * SUPER + i, ts(h, 512)],
                            in_=y_sb[:, i, ts(h, 512)],
                        )
                else:
                    for k in range(KB):
                        for h in range(2):
                            nc.tensor.matmul(
                                ps_y[:, ts(h, 512)],
                                lhsT=xT[:, k, :],
                                rhs=wq[:, k, ts(h, 512)],
                                start=(k == 0),
                                stop=(k == KB - 1),
                            )
                    epilogue(y_sb, i, j, ps_y, esc)

            def drain_y(j, y_sb):
                if j != NJ - 1:
                    nc.scalar.dma_start(
                        out=y_view[:, j * SUPER : (j + 1) * SUPER, :], in_=y_sb
                    )

            if not (apply_gamma or apply_beta):
                # ---- fast path: ternarize with super-1 stats interleaved;
                # the first super's two tiles then matmul k-interleaved so
                # consumption matches wq production with no PE stall.
                emit_tern(0, 2)
                emit_tern(2, 1)
                emit_tern(3, 2)
                fr1A = front_stats(x_supers[1], 0)
                emit_tern(5, 2)
                emit_tern(7, 1)
                fr1B = front_stats(x_supers[1], 1)
                frs[1] = [fr1A, fr1B]

                y_sb0 = y_pool.tile([P, SUPER, U], fp32)
                (xtA, sqA), (xtB, sqB) = fronts.pop(0)
                psA = ps_y_pool.tile([P, U], fp32, tag="ps_y")
                psB = ps_y_pool.tile([P, U], fp32, tag="ps_y")
                escA, escB = esc_for(sqA), esc_for(sqB)
                for k in range(KB):
                    for ps, xt in ((psA, xtA), (psB, xtB)):
                        for h in range(2):
                            nc.tensor.matmul(
                                ps[:, ts(h, 512)],
                                lhsT=xt[:, k, :],
                                rhs=wq[:, k, ts(h, 512)],
                                start=(k == 0),
                                stop=(k == KB - 1),
                            )
                epilogue(y_sb0, 0, 0, psA, escA)
                epilogue(y_sb0, 1, 0, psB, escB)
                drain_y(0, y_sb0)
            else:
                for q, ln in ((0, 2), (2, 1), (3, 2), (5, 2), (7, 1)):
                    emit_tern(q, ln)
                if apply_beta:
                    ps_beff = ps_y_pool.tile(
                        [P, U], fp32, tag="ps_y", name="ps_bf"
                    )
                    for k in range(KB):
                        for h in range(2):
                            nc.tensor.matmul(
                                ps_beff[0:1, ts(h, 512)],
                                lhsT=lb_sb[:, k : k + 1],
                                rhs=wq[:, k, ts(h, 512)],
                                start=(k == 0),
                                stop=(k == KB - 1),
                            )
                    beff = singles.tile([1, U], fp32)
                    nc.vector.tensor_scalar(
                        out=beff, in0=ps_beff[0:1, :], scalar1=bh128[0:1, 0:1],
                        scalar2=None, op0=OP.mult,
                    )
                    ps_b2 = ps_y_pool.tile([P, U], fp32, tag="ps_y")
                    ones_row = singles.tile([1, P], fp32)
                    nc.vector.memset(ones_row, 1.0)
                    for h in range(2):
                        nc.tensor.matmul(
                            ps_b2[:, ts(h, 512)], lhsT=ones_row,
                            rhs=beff[:, ts(h, 512)], start=True, stop=True,
                        )
                    beff128 = singles.tile([P, U], fp32)
                    nc.vector.tensor_copy(out=beff128, in_=ps_b2)
                    if apply_gamma:
                        for k in range(KB):
                            nc.vector.tensor_scalar(
                                out=wq[:, k, :], in0=wq[:, k, :],
                                scalar1=g_sb[:, k : k + 1], scalar2=None,
                                op0=OP.mult,
                            )
                frs[0] = [front_stats(x_supers[0], i) for i in range(SUPER)]
                frs[1] = [front_stats(x_supers[1], i) for i in range(SUPER)]
                f0 = [transpose_tile(fr) for fr in frs.pop(0)]
                y_sb0 = y_pool.tile([P, SUPER, U], fp32)
                for i in range(SUPER):
                    back_tile(f0[i], y_sb0, i, 0)
                drain_y(0, y_sb0)

            # super-1 transposes + copies; then x2 stats
            fronts[1] = [transpose_tile(fr) for fr in frs.pop(1)]
            x_supers[2] = issue_x(2, nc.sync)
            frs[2] = [front_stats(x_supers[2], i) for i in range(SUPER)]

            # ---- steady loop, per-tile interleave: M(j)i then T(j+1)i ----
            for j in range(1, NJ):
                y_sb = y_pool.tile([P, SUPER, U], fp32)
                xts = fronts.pop(j)
                nxt = [] if j + 1 < NJ else None
                for i in range(SUPER):
                    back_tile(xts[i], y_sb, i, j)
                    if nxt is not None:
                        nxt.append(transpose_tile(frs[j + 1][i]))
                if nxt is not None:
                    del frs[j + 1]
                    fronts[j + 1] = nxt
                drain_y(j, y_sb)
                if j + 2 < NJ:
                    x_supers[j + 2] = issue_x(j + 2, nc.sync)
                    frs[j + 2] = [
                        front_stats(x_supers[j + 2], i) for i in range(SUPER)
                    ]

    nc.compile()
    return nc


def _get_nc(apply_gamma: bool, apply_beta: bool):
    key = (apply_gamma, apply_beta)
    if key not in _NC_CACHE:
        _NC_CACHE[key] = _build(apply_gamma, apply_beta)
    return _NC_CACHE[key]


def _make_in_maps(x, w, g, lb, apply_gamma, apply_beta):
    xf = np.ascontiguousarray(x.reshape(B * S, D))
    in_maps = []
    for c in range(N_CORES):
        m = {
            "x": np.ascontiguousarray(xf[c * TOK : (c + 1) * TOK]),
            "weight": w,
        }
        if apply_gamma:
            m["ln_gamma"] = g
        if apply_beta:
            m["ln_beta"] = lb
        in_maps.append(m)
    return in_maps


def run(inputs, trace=False, tmpdir=None):
    """Shard, run on 8 cores, gather. Returns (y, BassKernelResults)."""
    from concourse.bass_utils import run_bass_kernel_spmd

    x = np.asarray(inputs["x"], dtype=np.float32)
    w = np.ascontiguousarray(np.asarray(inputs["weight"], dtype=np.float32))
    g = np.ascontiguousarray(np.asarray(inputs["ln_gamma"], dtype=np.float32))
    lb = np.ascontiguousarray(np.asarray(inputs["ln_beta"], dtype=np.float32))
    apply_gamma = not bool(np.all(g == 1.0))
    apply_beta = not bool(np.all(lb == 0.0))

    nc = _get_nc(apply_gamma, apply_beta)
    in_maps = _make_in_maps(x, w, g, lb, apply_gamma, apply_beta)
    res = run_bass_kernel_spmd(
        nc, in_maps, core_ids=list(range(N_CORES)), trace=trace, tmpdir=tmpdir
    )
    y = np.concatenate([r["y"] for r in res.results], axis=0)
    return y.reshape(B, S, U).astype(np.float32), res


def kernel(**inputs) -> np.ndarray:
    y, _ = run(inputs, trace=False)
    return y


# revision 47
# speedup vs baseline: 1.0161x; 1.0161x over previous
"""Trainium2 Bass kernel for nn_BitLinear (LayerNorm -> 1.58-bit BitLinear).

Math notes
----------
Reference computes, per the module:
    xn    = LN(x) * ln_gamma + ln_beta            (eps = 1e-3)
    beta  = mean(|W|);  w_q = clip(round(W / (beta + 1e-5)), -1, 1)
    gamma = max(|xn|)   (global absmax)
    xq    = clip(xn * 128 / gamma, -128 + 1e-5, 128 - 1e-5)
    y     = (xq @ w_q) * (gamma * beta / 128)

The gamma factor cancels exactly: (xn*128/gamma) @ w_q * (gamma*beta/128)
== (xn @ w_q) * beta.  The clip only affects elements within relative
7.8e-8 of the global absmax, changing them by ~1e-7 relative -- far below
f32 matmul roundoff.  So the kernel computes y = (LN(x) @ w_q) * beta,
which is fully data-parallel over tokens (no collectives needed).

w_q is ternary: w_q = sign(W) * 1[|W| > c] with c = 0.5*(beta+1e-5).
The kernel stores wq' = 0.5*w_q via ONE fused DVE op per k-group:
    wq = (|W| is_gt c) * sgnh,   sgnh = (W>=0)-0.5 in {-.5,+.5}
(scalar_tensor_tensor; the 2x is folded into the output scale 2*beta).
sgnh and |W| (f32, row-sums accumulated for beta) are computed while W
streams in.  All compares are f32: a bf16 compare would misclassify
~300 weights near the threshold (~2e-2 output error, at the budget).

LN normalization scale is folded into the epilogue: xn = (x - mu) in
bf16 (scale-invariant relative precision; the matmul is linear), and
esc[t] = rsqrt(var+eps)[t] * 2*beta scales each output row (ACT
scalar.mul with a [P,1] operand).

Sharding: data-parallel over the 32768 tokens, 4096 per core; weight
replicated (each core redundantly computes beta/w_q from the full W --
cheaper than a collective).

Schedule (measured: DMA starts ~8-10us after kernel start due to the
fixed engine preamble; the two HWDGE rings sustain ~410 GB/s combined
but share it unevenly; DVE f32 passes are ~0.7us, fused stt ~1.2us/k;
GPSIMD tensor ops are 2.5us AND stall concurrent DVE ops via SBUF
contention -- never used):
  * Ring q1 (sync/SP):    x0, W3a, W0, W1, W2, x1, x2, ...
    Ring q10 (scalar/ACT): W3b, W4..W7, y0, y1, ...
    (W3 split across rings balances their drain at ~23.5us; x supers
    queue behind W by ring FIFO so they cannot steal prologue
    bandwidth.)
  * W prep per chunk, in landing order: DVE extracts sgnh, then |W| in
    place via one fused pass for early q1 chunks, ACT Abs+accum for q10
    chunks; the LAST chunk on each ring (2 and 7) writes |W| to scratch
    instead (not in place) so its sgnh runs concurrently and the
    beta -> c chain closes ~1us after the last W byte.
  * The PE would idle >12us waiting for W; idle >3.4us drops its clock
    to 1.2 GHz (HAM gate).  Dummy identity transposes keep it busy+warm
    until real work arrives.
  * Ternarize emits as k-grouped fused ops with super-1 stats
    interleaved; the first super's matmuls interleave its two tiles
    k-by-k so consumption (~1.7us/k) matches wq production (~1.2us/k)
    with no PE stall.
  * Steady loop per tile: M(j)i then T(j+1)i -- the xT PSUM->SBUF copy
    for super j+1 runs on DVE during back(j)'s matmuls, so the PE never
    waits on copies.
  * Final super runs h-major with per-half drains on both rings, so the
    post-matmul tail is one 256 KiB transfer deep per ring.
"""

import numpy as np

B, S, D, U = 4, 8192, 1024, 1024
N_CORES = 8
TOK = (B * S) // N_CORES  # 4096 tokens per core
P = 128
KB = D // P               # 8 contraction blocks
NTILES = TOK // P         # 32 token tiles per core
SUPER = 2                 # token tiles per DMA transfer (1 MiB chunks)
NJ = NTILES // SUPER      # 16 super-tiles
N_DUMMY1 = 185            # PE warmup transposes before T0
N_DUMMY2 = 140            # ... between T0 and the ones-matmul
LN_EPS = 1e-3
EPS = 1e-5

_NC_CACHE = {}


def _build(apply_gamma: bool, apply_beta: bool):
    """Build the single-core Bass program (SPMD: same NEFF on all 8 cores)."""
    import concourse.bacc as bacc
    import concourse.bass_isa as bass_isa
    import concourse.mybir as mybir
    import concourse.tile as tile
    from concourse.bass import ts
    from concourse.masks import make_identity

    fp32 = mybir.dt.float32
    bf16 = mybir.dt.bfloat16
    AF = mybir.ActivationFunctionType
    OP = mybir.AluOpType

    nc = bacc.Bacc()
    x_h = nc.dram_tensor("x", [TOK, D], fp32, kind="ExternalInput")
    w_h = nc.dram_tensor("weight", [D, U], fp32, kind="ExternalInput")
    g_h = (
        nc.dram_tensor("ln_gamma", [D], fp32, kind="ExternalInput")
        if apply_gamma
        else None
    )
    lb_h = (
        nc.dram_tensor("ln_beta", [D], fp32, kind="ExternalInput")
        if apply_beta
        else None
    )
    y_h = nc.dram_tensor("y", [TOK, U], fp32, kind="ExternalOutput")

    DVE_ABS = (0, 1, 3, 4)   # chunks whose fused |W|+accum rides DVE
    SCRATCH_ABS = (2, 5, 6, 7)  # ACT abs to scratch (no sgnh ordering dep)

    with tile.TileContext(nc) as tc:
        with (
            tc.tile_pool(name="singles", bufs=1) as singles,
            tc.tile_pool(name="xin", bufs=4) as xin_pool,
            tc.tile_pool(name="xn", bufs=6) as xn_pool,
            tc.tile_pool(name="xt", bufs=8) as xt_pool,
            tc.tile_pool(name="yout", bufs=3) as y_pool,
            tc.tile_pool(name="stats", bufs=6) as stats_pool,
            tc.tile_pool(name="ps_t", bufs=4, space="PSUM") as ps_t_pool,
            tc.tile_pool(name="ps_y", bufs=2, space="PSUM") as ps_y_pool,
        ):
            # ---- constants ----
            ident = singles.tile([P, P], bf16)
            make_identity(nc, ident)
            eps_t = singles.tile([P, 1], fp32)
            nc.vector.memset(eps_t, LN_EPS)
            ones_f32 = singles.tile([P, P], fp32)
            nc.vector.memset(ones_f32, 1.0)

            # ---- DMA issue order defines ring FIFO order ----
            w_view = w_h[:, :].rearrange("(ko ki) u -> ki ko u", ki=P)
            x_view = x_h[:, :].rearrange("(o p) d -> p o d", p=P)
            y_view = y_h[:, :].rearrange("(o p) u -> p o u", p=P)

            def issue_x(j, eng):
                x_sb = xin_pool.tile([P, SUPER, D], fp32, name="x_sb")
                eng.dma_start(
                    out=x_sb, in_=x_view[:, j * SUPER : (j + 1) * SUPER, :]
                )
                return x_sb

            w_sb = singles.tile([P, KB, U], fp32)
            x_supers = {0: issue_x(0, nc.sync)}
            # q1: x0, W3a, W0, W1, W2, x1 ...   q10: W3b, W4..W7, y ...
            nc.sync.dma_start(out=w_sb[:, 3, 0:512], in_=w_view[:, 3, 0:512])
            nc.scalar.dma_start(
                out=w_sb[:, 3, 512:1024], in_=w_view[:, 3, 512:1024]
            )
            for k in (0, 1, 2):
                nc.sync.dma_start(out=w_sb[:, k, :], in_=w_view[:, k, :])
            for k in (4, 5, 6, 7):
                nc.scalar.dma_start(out=w_sb[:, k, :], in_=w_view[:, k, :])
            x_supers[1] = issue_x(1, nc.sync)

            if apply_gamma:
                g_sb = singles.tile([P, KB], fp32)
                nc.scalar.dma_start(
                    out=g_sb, in_=g_h[:].rearrange("(ko ki) -> ki ko", ki=P)
                )
            if apply_beta:
                lb_f32 = singles.tile([P, KB], fp32)
                nc.scalar.dma_start(
                    out=lb_f32, in_=lb_h[:].rearrange("(ko ki) -> ki ko", ki=P)
                )
                lb_sb = singles.tile([P, KB], bf16)
                nc.vector.tensor_copy(out=lb_sb, in_=lb_f32)

            # ---- W prep: sgnh (sign), then |W| + row-sums for beta ----
            sgnh = singles.tile([P, KB, U], bf16)
            asum = singles.tile([P, KB], fp32)
            scratch = singles.tile([P, 4, U], fp32)  # |W| for ACT chunks
            abs_src = {}  # k -> AP holding |W| for the ternarize compare

            def emit_sgnh(k):
                # (W>=0)-0.5 in {-.5,+.5}, exact in bf16
                nc.vector.tensor_scalar(
                    out=sgnh[:, k, :], in0=w_sb[:, k, :], scalar1=0.0,
                    scalar2=0.5, op0=OP.is_ge, op1=OP.subtract,
                )
                if apply_gamma and not apply_beta:
                    # fold ln_gamma rows in (the beff path needs raw w_q, so
                    # the combined variant applies gamma later instead)
                    nc.vector.tensor_scalar(
                        out=sgnh[:, k, :], in0=sgnh[:, k, :],
                        scalar1=g_sb[:, k : k + 1], scalar2=None, op0=OP.mult,
                    )

            def emit_abs(k):
                if k in SCRATCH_ABS:
                    # NOT in place: sgnh (DVE) runs concurrently with this
                    # ACT pass -- critical for the last chunk on each ring
                    i = SCRATCH_ABS.index(k)
                    nc.scalar.activation(
                        out=scratch[:, i, :], in_=w_sb[:, k, :], func=AF.Abs,
                        accum_out=asum[:, k : k + 1],
                    )
                    abs_src[k] = scratch[:, i : i + 1, :]
                elif k in DVE_ABS:
                    # |W| = (2W) * sgnh exactly, row-sum accumulated: one
                    # fused DVE pass, in place
                    nc.vector.scalar_tensor_tensor(
                        out=w_sb[:, k, :], in0=w_sb[:, k, :], scalar=2.0,
                        in1=sgnh[:, k, :], op0=OP.mult, op1=OP.mult,
                        accum_out=asum[:, k : k + 1],
                    )
                    abs_src[k] = w_sb[:, k : k + 1, :]
                else:
                    nc.scalar.activation(
                        out=w_sb[:, k, :], in_=w_sb[:, k, :], func=AF.Abs,
                        accum_out=asum[:, k : k + 1],
                    )
                    abs_src[k] = w_sb[:, k : k + 1, :]

            # ---- LN stats on DVE; the normalize pass rides ACT ----
            def front_stats(x_sb, i):
                xt_ = x_sb[:, i, :]
                st = stats_pool.tile([P, 2, 6], fp32, tag="st")
                xr = xt_.rearrange("p (n f) -> p n f", f=512)
                nc.vector.bn_stats(out=st[:, 0, :], in_=xr[:, 0, :])
                nc.vector.bn_stats(out=st[:, 1, :], in_=xr[:, 1, :])
                mv = stats_pool.tile([P, 2], fp32, tag="mv")
                nc.vector.bn_aggr(out=mv, in_=st)
                nb = stats_pool.tile([P, 1], fp32, tag="nb")
                nc.vector.tensor_scalar(
                    out=nb, in0=mv[:, 0:1], scalar1=-1.0, scalar2=None,
                    op0=OP.mult,
                )
                # xn = x - mu (bf16); rsqrt scale folds into the epilogue
                xn = xn_pool.tile([P, D], bf16)
                nc.scalar.activation(
                    out=xn, in_=xt_, func=AF.Identity, bias=nb, scale=1.0
                )
                # sq = sqrt(var + eps) (tiny, ACT)
                sq = stats_pool.tile([P, 1], fp32, tag="sq")
                nc.scalar.activation(
                    out=sq, in_=mv[:, 1:2], func=AF.Sqrt, bias=eps_t, scale=1.0
                )
                return xn, sq

            # tiny ACT table warm-ups: they run in ACT's natural idle window
            # (after descriptor-gen, before the first chunk lands) so the
            # Abs/Sqrt/Copy tables aren't loaded on the critical path later
            warm = singles.tile([P, 3], fp32)
            nc.scalar.activation(out=warm[:, 0:1], in_=eps_t, func=AF.Abs)
            nc.scalar.activation(
                out=warm[:, 1:2], in_=eps_t, func=AF.Sqrt, bias=eps_t, scale=1.0
            )
            nc.scalar.copy(out=warm[:, 2:3], in_=eps_t)

            # W prep in expected landing order (q10: W3b@12, W4@15, W5@19,
            # W6@22, W7@23.5; q1: x0@14, W3a@16, W0@19, W1@21, W2@23.5).
            # ACT chunks write |W| to scratch so they have no sgnh WAR dep;
            # DVE chunks fuse sgnh-mult+accum in one in-place pass.
            emit_sgnh(4)
            emit_abs(4)
            frs = {0: [front_stats(x_supers[0], i) for i in range(SUPER)]}
            emit_sgnh(3)
            emit_abs(3)
            emit_abs(5)
            emit_sgnh(0)
            emit_abs(0)
            emit_abs(6)
            emit_sgnh(1)
            emit_abs(1)

            # ---- PE warmup dummies (keep the HAM clock at 2.4 GHz) ----
            ps_dummy = ps_t_pool.tile([P, KB, P], bf16, tag="ps_t", name="ps_d")
            for i in range(N_DUMMY1):
                nc.tensor.transpose(ps_dummy[:, i % KB, :], ident, ident)

            # ---- transposes + copies ----
            def transpose_tile(fr, copy_eng=None):
                xn, sq = fr
                ps_xt = ps_t_pool.tile([P, KB, P], bf16, tag="ps_t")
                for k in range(KB):
                    nc.tensor.transpose(ps_xt[:, k, :], xn[:, ts(k, P)], ident)
                xT = xt_pool.tile([P, KB, P], bf16)
                (copy_eng or nc.vector.tensor_copy)(out=xT, in_=ps_xt)
                return (xT, sq)

            def transpose_tile_dma(fr, eng):
                # steady state: the DMA xbar does the [tok,d]->[d,tok]
                # transpose (out[di,k,t] = in[t, k*128+di]), freeing the PE
                # of ~14us of transposes and the DVE of the PSUM copies
                xn, sq = fr
                xT = xt_pool.tile([P, KB, P], bf16)
                eng.dma_start_transpose(xT[:, :, :], xn[:, :])
                return (xT, sq)

            fronts = {0: [transpose_tile(fr) for fr in frs.pop(0)]}

            # W prep tail: the last-landing chunks (scratch-abs on ACT runs
            # concurrently with their sgnh on DVE)
            emit_abs(2)
            emit_abs(7)
            emit_sgnh(5)
            emit_sgnh(6)
            emit_sgnh(2)
            emit_sgnh(7)
            asum1 = singles.tile([P, 1], fp32)
            nc.vector.tensor_reduce(
                out=asum1, in_=asum, axis=mybir.AxisListType.X, op=OP.add
            )

            for i in range(N_DUMMY2):
                nc.tensor.transpose(ps_dummy[:, i % KB, :], ident, ident)

            # cross-partition total broadcast to all partitions in ONE matmul
            ps_tot = ps_y_pool.tile([P, U], fp32, tag="ps_y", name="ps_tot")
            nc.tensor.matmul(
                ps_tot[:, 0:1], lhsT=ones_f32, rhs=asum1, start=True, stop=True
            )
            t128 = singles.tile([P, 1], fp32)
            nc.vector.tensor_copy(out=t128, in_=ps_tot[:, 0:1])
            # c = (beta+EPS)/2 ;  output scale 2*beta (wq holds 0.5*w_q)
            c128 = singles.tile([P, 1], fp32)
            nc.vector.tensor_scalar(
                out=c128, in0=t128, scalar1=0.5 / (D * U), scalar2=0.5 * EPS,
                op0=OP.mult, op1=OP.add,
            )
            bh128 = singles.tile([P, 1], fp32)
            nc.vector.tensor_scalar(
                out=bh128, in0=t128, scalar1=2.0 / (D * U), scalar2=None,
                op0=OP.mult,
            )

            # ---- ternarize: fused (|W| is_gt c) * sgnh per k-group ----
            wq = singles.tile([P, KB, U], bf16)  # holds 0.5*w_q (*gamma)

            def emit_tern(k0, klen):
                if k0 in SCRATCH_ABS:
                    i0 = SCRATCH_ABS.index(k0)
                    assert all(
                        SCRATCH_ABS.index(k0 + q) == i0 + q for q in range(klen)
                    )
                    src = scratch[:, i0 : i0 + klen, :]
                else:
                    assert all(k0 + q in DVE_ABS for q in range(klen))
                    src = w_sb[:, k0 : k0 + klen, :]
                nc.vector.scalar_tensor_tensor(
                    out=wq[:, k0 : k0 + klen, :], in0=src, scalar=c128,
                    in1=sgnh[:, k0 : k0 + klen, :], op0=OP.is_gt, op1=OP.mult,
                )

            beff128 = None

            # ---- back side ----
            def esc_for(sq):
                # esc = rsqrt(var+eps) * 2*beta, per token (tiny DVE chain)
                esc = stats_pool.tile([P, 1], fp32, tag="esc")
                nc.vector.reciprocal(esc, sq)
                nc.vector.tensor_scalar(
                    out=esc, in0=esc, scalar1=bh128, scalar2=None, op0=OP.mult
                )
                return esc

            def epilogue(y_sb, i, j, ps_y, esc, h=None):
                sl = slice(None) if h is None else ts(h, 512)
                nc.scalar.mul(out=y_sb[:, i, sl], in_=ps_y[:, sl], mul=esc)
                if apply_beta:
                    nc.vector.tensor_tensor(
                        y_sb[:, i, sl], y_sb[:, i, sl], beff128[:, sl], OP.add
                    )

            def back_tile(xt_sq, y_sb, i, j):
                xT, sq = xt_sq
                last = j == NJ - 1
                ps_y = ps_y_pool.tile([P, U], fp32, tag="ps_y")
                esc = esc_for(sq)
                if last:
                    # h-major: each half's epilogue + 256 KiB drain starts as
                    # soon as that half's accumulation closes (short tail)
                    for h in range(2):
                        for k in range(KB):
                            nc.tensor.matmul(
                                ps_y[:, ts(h, 512)],
                                lhsT=xT[:, k, :],
                                rhs=wq[:, k, ts(h, 512)],
                                start=(k == 0),
                                stop=(k == KB - 1),
                            )
                        epilogue(y_sb, i, j, ps_y, esc, h)
                        eng = nc.scalar if h == 0 else nc.sync
                        eng.dma_start(
                            out=y_view[:, j * SUPER + i, ts(h, 512)],
                            in_=y_sb[:, i, ts(h, 512)],
                        )
                else:
                    for k in range(KB):
                        for h in range(2):
                            nc.tensor.matmul(
                                ps_y[:, ts(h, 512)],
                                lhsT=xT[:, k, :],
                                rhs=wq[:, k, ts(h, 512)],
                                start=(k == 0),
                                stop=(k == KB - 1),
                            )
                    epilogue(y_sb, i, j, ps_y, esc)

            def drain_y(j, y_sb):
                if j != NJ - 1:
                    nc.scalar.dma_start(
                        out=y_view[:, j * SUPER : (j + 1) * SUPER, :], in_=y_sb
                    )

            if not (apply_gamma or apply_beta):
                # ---- fast path: ternarize with super-1 stats interleaved;
                # the first super's two tiles then matmul k-interleaved so
                # consumption matches wq production with no PE stall.
                emit_tern(0, 2)
                emit_tern(2, 1)
                emit_tern(3, 2)
                fr1A = front_stats(x_supers[1], 0)
                emit_tern(5, 2)
                emit_tern(7, 1)
                fr1B = front_stats(x_supers[1], 1)
                frs[1] = [fr1A, fr1B]

                y_sb0 = y_pool.tile([P, SUPER, U], fp32)
                (xtA, sqA), (xtB, sqB) = fronts.pop(0)
                psA = ps_y_pool.tile([P, U], fp32, tag="ps_y")
                psB = ps_y_pool.tile([P, U], fp32, tag="ps_y")
                escA, escB = esc_for(sqA), esc_for(sqB)
                for k in range(KB):
                    for ps, xt in ((psA, xtA), (psB, xtB)):
                        for h in range(2):
                            nc.tensor.matmul(
                                ps[:, ts(h, 512)],
                                lhsT=xt[:, k, :],
                                rhs=wq[:, k, ts(h, 512)],
                                start=(k == 0),
                                stop=(k == KB - 1),
                            )
                epilogue(y_sb0, 0, 0, psA, escA)
                epilogue(y_sb0, 1, 0, psB, escB)
                drain_y(0, y_sb0)
            else:
                for q, ln in ((0, 2), (2, 1), (3, 2), (5, 2), (7, 1)):
                    emit_tern(q, ln)
                if apply_beta:
                    ps_beff = ps_y_pool.tile(
                        [P, U], fp32, tag="ps_y", name="ps_bf"
                    )
                    for k in range(KB):
                        for h in range(2):
                            nc.tensor.matmul(
                                ps_beff[0:1, ts(h, 512)],
                                lhsT=lb_sb[:, k : k + 1],
                                rhs=wq[:, k, ts(h, 512)],
                                start=(k == 0),
                                stop=(k == KB - 1),
                            )
                    beff = singles.tile([1, U], fp32)
                    nc.vector.tensor_scalar(
                        out=beff, in0=ps_beff[0:1, :], scalar1=bh128[0:1, 0:1],
                        scalar2=None, op0=OP.mult,
                    )
                    ps_b2 = ps_y_pool.tile([P, U], fp32, tag="ps_y")
                    ones_row = singles.tile([1, P], fp32)
                    nc.vector.memset(ones_row, 1.0)
                    for h in range(2):
                        nc.tensor.matmul(
                            ps_b2[:, ts(h, 512)], lhsT=ones_row,
                            rhs=beff[:, ts(h, 512)], start=True, stop=True,
                        )
                    beff128 = singles.tile([P, U], fp32)
                    nc.vector.tensor_copy(out=beff128, in_=ps_b2)
                    if apply_gamma:
                        for k in range(KB):
                            nc.vector.tensor_scalar(
                                out=wq[:, k, :], in0=wq[:, k, :],
                                scalar1=g_sb[:, k : k + 1], scalar2=None,
                                op0=OP.mult,
                            )
                frs[0] = [front_stats(x_supers[0], i) for i in range(SUPER)]
                frs[1] = [front_stats(x_supers[1], i) for i in range(SUPER)]
                f0 = [transpose_tile(fr) for fr in frs.pop(0)]
                y_sb0 = y_pool.tile([P, SUPER, U], fp32)
                for i in range(SUPER):
                    back_tile(f0[i], y_sb0, i, 0)
                drain_y(0, y_sb0)

            # super-1 transposes + copies; then x2 stats
            fronts[1] = [transpose_tile(fr) for fr in frs.pop(1)]
            x_supers[2] = issue_x(2, nc.sync)
            frs[2] = [front_stats(x_supers[2], i) for i in range(SUPER)]

            # ---- steady loop, per-tile interleave: M(j)i then T(j+1)i ----
            for j in range(1, NJ):
                y_sb = y_pool.tile([P, SUPER, U], fp32)
                xts = fronts.pop(j)
                nxt = [] if j + 1 < NJ else None
                for i in range(SUPER):
                    back_tile(xts[i], y_sb, i, j)
                    if nxt is not None:
                        nxt.append(transpose_tile(frs[j + 1][i]))
                if nxt is not None:
                    del frs[j + 1]
                    fronts[j + 1] = nxt
                drain_y(j, y_sb)
                if j + 2 < NJ:
                    x_supers[j + 2] = issue_x(j + 2, nc.sync)
                    frs[j + 2] = [
                        front_stats(x_supers[j + 2], i) for i in range(SUPER)
                    ]

    nc.compile()
    return nc


def _get_nc(apply_gamma: bool, apply_beta: bool):
    key = (apply_gamma, apply_beta)
    if key not in _NC_CACHE:
        _NC_CACHE[key] = _build(apply_gamma, apply_beta)
    return _NC_CACHE[key]


def _make_in_maps(x, w, g, lb, apply_gamma, apply_beta):
    xf = np.ascontiguousarray(x.reshape(B * S, D))
    in_maps = []
    for c in range(N_CORES):
        m = {
            "x": np.ascontiguousarray(xf[c * TOK : (c + 1) * TOK]),
            "weight": w,
        }
        if apply_gamma:
            m["ln_gamma"] = g
        if apply_beta:
            m["ln_beta"] = lb
        in_maps.append(m)
    return in_maps


def run(inputs, trace=False, tmpdir=None):
    """Shard, run on 8 cores, gather. Returns (y, BassKernelResults)."""
    from concourse.bass_utils import run_bass_kernel_spmd

    x = np.asarray(inputs["x"], dtype=np.float32)
    w = np.ascontiguousarray(np.asarray(inputs["weight"], dtype=np.float32))
    g = np.ascontiguousarray(np.asarray(inputs["ln_gamma"], dtype=np.float32))
    lb = np.ascontiguousarray(np.asarray(inputs["ln_beta"], dtype=np.float32))
    apply_gamma = not bool(np.all(g == 1.0))
    apply_beta = not bool(np.all(lb == 0.0))

    nc = _get_nc(apply_gamma, apply_beta)
    in_maps = _make_in_maps(x, w, g, lb, apply_gamma, apply_beta)
    res = run_bass_kernel_spmd(
        nc, in_maps, core_ids=list(range(N_CORES)), trace=trace, tmpdir=tmpdir
    )
    y = np.concatenate([r["y"] for r in res.results], axis=0)
    return y.reshape(B, S, U).astype(np.float32), res


def kernel(**inputs) -> np.ndarray:
    y, _ = run(inputs, trace=False)
    return y


# revision 52
# speedup vs baseline: 1.0332x; 1.0169x over previous
"""Trainium2 Bass kernel for nn_BitLinear (LayerNorm -> 1.58-bit BitLinear).

Math notes
----------
Reference computes, per the module:
    xn    = LN(x) * ln_gamma + ln_beta            (eps = 1e-3)
    beta  = mean(|W|);  w_q = clip(round(W / (beta + 1e-5)), -1, 1)
    gamma = max(|xn|)   (global absmax)
    xq    = clip(xn * 128 / gamma, -128 + 1e-5, 128 - 1e-5)
    y     = (xq @ w_q) * (gamma * beta / 128)

The gamma factor cancels exactly: (xn*128/gamma) @ w_q * (gamma*beta/128)
== (xn @ w_q) * beta.  The clip only affects elements within relative
7.8e-8 of the global absmax, changing them by ~1e-7 relative -- far below
f32 matmul roundoff.  So the kernel computes y = (LN(x) @ w_q) * beta,
which is fully data-parallel over tokens (no collectives needed).

w_q is ternary: w_q = sign(W) * 1[|W| > c] with c = 0.5*(beta+1e-5).
The kernel stores wq' = 0.5*w_q via ONE fused DVE op per k-group:
    wq = (|W| is_gt c) * sgnh,   sgnh = (W>=0)-0.5 in {-.5,+.5}
(scalar_tensor_tensor; the 2x is folded into the output scale 2*beta).
sgnh and |W| (f32, row-sums accumulated for beta) are computed while W
streams in.  All compares are f32: a bf16 compare would misclassify
~300 weights near the threshold (~2e-2 output error, at the budget).

LN normalization scale is folded into the epilogue: xn = (x - mu) in
bf16 (scale-invariant relative precision; the matmul is linear), and
esc[t] = rsqrt(var+eps)[t] * 2*beta scales each output row (ACT
scalar.mul with a [P,1] operand).

Sharding: data-parallel over the 32768 tokens, 4096 per core; weight
replicated (each core redundantly computes beta/w_q from the full W --
cheaper than a collective).

Schedule (measured: DMA starts ~8-10us after kernel start due to the
fixed engine preamble; the two HWDGE rings sustain ~410 GB/s combined
but share it unevenly; DVE f32 passes are ~0.7us, fused stt ~1.2us/k;
GPSIMD tensor ops are 2.5us AND stall concurrent DVE ops via SBUF
contention -- never used):
  * Ring q1 (sync/SP):    x0, W3a, W0, W1, W2, x1, x2, ...
    Ring q10 (scalar/ACT): W3b, W4..W7, y0, y1, ...
    (W3 split across rings balances their drain at ~23.5us; x supers
    queue behind W by ring FIFO so they cannot steal prologue
    bandwidth.)
  * W prep per chunk, in landing order: DVE extracts sgnh, then |W| in
    place via one fused pass for early q1 chunks, ACT Abs+accum for q10
    chunks; the LAST chunk on each ring (2 and 7) writes |W| to scratch
    instead (not in place) so its sgnh runs concurrently and the
    beta -> c chain closes ~1us after the last W byte.
  * The PE would idle >12us waiting for W; idle >3.4us drops its clock
    to 1.2 GHz (HAM gate).  Dummy identity transposes keep it busy+warm
    until real work arrives.
  * Ternarize emits as k-grouped fused ops with super-1 stats
    interleaved; the first super's matmuls interleave its two tiles
    k-by-k so consumption (~1.7us/k) matches wq production (~1.2us/k)
    with no PE stall.
  * Steady loop per tile: M(j)i then T(j+1)i -- the xT PSUM->SBUF copy
    for super j+1 runs on DVE during back(j)'s matmuls, so the PE never
    waits on copies.
  * Final super runs h-major with per-half drains on both rings, so the
    post-matmul tail is one 256 KiB transfer deep per ring.
"""

import numpy as np

B, S, D, U = 4, 8192, 1024, 1024
N_CORES = 8
TOK = (B * S) // N_CORES  # 4096 tokens per core
P = 128
KB = D // P               # 8 contraction blocks
NTILES = TOK // P         # 32 token tiles per core
SUPER = 2                 # token tiles per DMA transfer (1 MiB chunks)
NJ = NTILES // SUPER      # 16 super-tiles
N_DUMMY1 = 185            # PE warmup transposes before T0
N_DUMMY2 = 140            # ... between T0 and the ones-matmul
LN_EPS = 1e-3
EPS = 1e-5

_NC_CACHE = {}


def _build(apply_gamma: bool, apply_beta: bool):
    """Build the single-core Bass program (SPMD: same NEFF on all 8 cores)."""
    import concourse.bacc as bacc
    import concourse.bass_isa as bass_isa
    import concourse.mybir as mybir
    import concourse.tile as tile
    from concourse.bass import ts
    from concourse.masks import make_identity

    fp32 = mybir.dt.float32
    bf16 = mybir.dt.bfloat16
    AF = mybir.ActivationFunctionType
    OP = mybir.AluOpType

    nc = bacc.Bacc()
    x_h = nc.dram_tensor("x", [TOK, D], fp32, kind="ExternalInput")
    w_h = nc.dram_tensor("weight", [D, U], fp32, kind="ExternalInput")
    g_h = (
        nc.dram_tensor("ln_gamma", [D], fp32, kind="ExternalInput")
        if apply_gamma
        else None
    )
    lb_h = (
        nc.dram_tensor("ln_beta", [D], fp32, kind="ExternalInput")
        if apply_beta
        else None
    )
    y_h = nc.dram_tensor("y", [TOK, U], fp32, kind="ExternalOutput")

    DVE_ABS = (0, 1)      # chunks whose fused |W|+accum rides DVE
    SCRATCH_ABS = (2, 7)  # last chunk per ring: ACT abs to scratch

    with tile.TileContext(nc) as tc:
        with (
            tc.tile_pool(name="singles", bufs=1) as singles,
            tc.tile_pool(name="xin", bufs=4) as xin_pool,
            tc.tile_pool(name="xn", bufs=6) as xn_pool,
            tc.tile_pool(name="xt", bufs=8) as xt_pool,
            tc.tile_pool(name="yout", bufs=3) as y_pool,
            tc.tile_pool(name="stats", bufs=6) as stats_pool,
            tc.tile_pool(name="ps_t", bufs=4, space="PSUM") as ps_t_pool,
            tc.tile_pool(name="ps_y", bufs=2, space="PSUM") as ps_y_pool,
        ):
            # ---- constants ----
            ident = singles.tile([P, P], bf16)
            make_identity(nc, ident)
            eps_t = singles.tile([P, 1], fp32)
            nc.vector.memset(eps_t, LN_EPS)
            ones_f32 = singles.tile([P, P], fp32)
            nc.vector.memset(ones_f32, 1.0)

            # ---- DMA issue order defines ring FIFO order ----
            w_view = w_h[:, :].rearrange("(ko ki) u -> ki ko u", ki=P)
            x_view = x_h[:, :].rearrange("(o p) d -> p o d", p=P)
            y_view = y_h[:, :].rearrange("(o p) u -> p o u", p=P)

            def issue_x(j, eng):
                x_sb = xin_pool.tile([P, SUPER, D], fp32, name="x_sb")
                eng.dma_start(
                    out=x_sb, in_=x_view[:, j * SUPER : (j + 1) * SUPER, :]
                )
                return x_sb

            w_sb = singles.tile([P, KB, U], fp32)
            x_supers = {0: issue_x(0, nc.sync)}
            # q1: x0, W3a, W0, W1, W2, x1 ...   q10: W3b, W4..W7, y ...
            nc.sync.dma_start(out=w_sb[:, 3, 0:512], in_=w_view[:, 3, 0:512])
            nc.scalar.dma_start(
                out=w_sb[:, 3, 512:1024], in_=w_view[:, 3, 512:1024]
            )
            for k in (0, 1, 2):
                nc.sync.dma_start(out=w_sb[:, k, :], in_=w_view[:, k, :])
            for k in (4, 5, 6, 7):
                nc.scalar.dma_start(out=w_sb[:, k, :], in_=w_view[:, k, :])
            x_supers[1] = issue_x(1, nc.sync)

            if apply_gamma:
                g_sb = singles.tile([P, KB], fp32)
                nc.scalar.dma_start(
                    out=g_sb, in_=g_h[:].rearrange("(ko ki) -> ki ko", ki=P)
                )
            if apply_beta:
                lb_f32 = singles.tile([P, KB], fp32)
                nc.scalar.dma_start(
                    out=lb_f32, in_=lb_h[:].rearrange("(ko ki) -> ki ko", ki=P)
                )
                lb_sb = singles.tile([P, KB], bf16)
                nc.vector.tensor_copy(out=lb_sb, in_=lb_f32)

            # ---- W prep: sgnh (sign), then |W| + row-sums for beta ----
            sgnh = singles.tile([P, KB, U], bf16)
            asum = singles.tile([P, KB], fp32)
            scratch = singles.tile([P, 2, U], fp32)  # |W| for chunks 2 and 7
            abs_src = {}  # k -> AP holding |W| for the ternarize compare

            def emit_sgnh(k):
                # (W>=0)-0.5 in {-.5,+.5}, exact in bf16
                nc.vector.tensor_scalar(
                    out=sgnh[:, k, :], in0=w_sb[:, k, :], scalar1=0.0,
                    scalar2=0.5, op0=OP.is_ge, op1=OP.subtract,
                )
                if apply_gamma and not apply_beta:
                    # fold ln_gamma rows in (the beff path needs raw w_q, so
                    # the combined variant applies gamma later instead)
                    nc.vector.tensor_scalar(
                        out=sgnh[:, k, :], in0=sgnh[:, k, :],
                        scalar1=g_sb[:, k : k + 1], scalar2=None, op0=OP.mult,
                    )

            def emit_abs(k):
                if k in SCRATCH_ABS:
                    # NOT in place: sgnh (DVE) runs concurrently with this
                    # ACT pass -- critical for the last chunk on each ring
                    i = SCRATCH_ABS.index(k)
                    nc.scalar.activation(
                        out=scratch[:, i, :], in_=w_sb[:, k, :], func=AF.Abs,
                        accum_out=asum[:, k : k + 1],
                    )
                    abs_src[k] = scratch[:, i : i + 1, :]
                elif k in DVE_ABS:
                    # |W| = (2W) * sgnh exactly, row-sum accumulated: one
                    # fused DVE pass, in place
                    nc.vector.scalar_tensor_tensor(
                        out=w_sb[:, k, :], in0=w_sb[:, k, :], scalar=2.0,
                        in1=sgnh[:, k, :], op0=OP.mult, op1=OP.mult,
                        accum_out=asum[:, k : k + 1],
                    )
                    abs_src[k] = w_sb[:, k : k + 1, :]
                else:
                    nc.scalar.activation(
                        out=w_sb[:, k, :], in_=w_sb[:, k, :], func=AF.Abs,
                        accum_out=asum[:, k : k + 1],
                    )
                    abs_src[k] = w_sb[:, k : k + 1, :]

            # ---- LN stats on DVE; the normalize pass rides ACT ----
            def front_stats(x_sb, i):
                xt_ = x_sb[:, i, :]
                st = stats_pool.tile([P, 2, 6], fp32, tag="st")
                xr = xt_.rearrange("p (n f) -> p n f", f=512)
                nc.vector.bn_stats(out=st[:, 0, :], in_=xr[:, 0, :])
                nc.vector.bn_stats(out=st[:, 1, :], in_=xr[:, 1, :])
                mv = stats_pool.tile([P, 2], fp32, tag="mv")
                nc.vector.bn_aggr(out=mv, in_=st)
                nb = stats_pool.tile([P, 1], fp32, tag="nb")
                nc.vector.tensor_scalar(
                    out=nb, in0=mv[:, 0:1], scalar1=-1.0, scalar2=None,
                    op0=OP.mult,
                )
                # xn = x - mu (bf16); rsqrt scale folds into the epilogue
                xn = xn_pool.tile([P, D], bf16)
                nc.scalar.activation(
                    out=xn, in_=xt_, func=AF.Identity, bias=nb, scale=1.0
                )
                # sq = sqrt(var + eps) (tiny, ACT)
                sq = stats_pool.tile([P, 1], fp32, tag="sq")
                nc.scalar.activation(
                    out=sq, in_=mv[:, 1:2], func=AF.Sqrt, bias=eps_t, scale=1.0
                )
                return xn, sq

            # W prep in expected landing order (q10: W3b@12, W4@15, W5@19,
            # W6@22, W7@23.5; q1: x0@14, W3a@16, W0@19, W1@21, W2@23.5).
            # DVE chunks fuse sgnh-mult+accum in one in-place pass; the
            # last chunk per ring writes |W| to scratch so its sgnh (DVE)
            # runs concurrently with the ACT abs.
            emit_sgnh(4)
            emit_abs(4)
            frs = {0: [front_stats(x_supers[0], i) for i in range(SUPER)]}
            emit_sgnh(3)
            emit_abs(3)
            emit_sgnh(5)
            emit_abs(5)
            emit_sgnh(0)
            emit_abs(0)
            emit_sgnh(1)
            emit_abs(1)
            emit_sgnh(6)
            emit_abs(6)

            # ---- PE warmup dummies (keep the HAM clock at 2.4 GHz) ----
            ps_dummy = ps_t_pool.tile([P, KB, P], bf16, tag="ps_t", name="ps_d")
            for i in range(N_DUMMY1):
                nc.tensor.transpose(ps_dummy[:, i % KB, :], ident, ident)

            # ---- transposes + copies ----
            def transpose_tile(fr, copy_eng=None):
                xn, sq = fr
                ps_xt = ps_t_pool.tile([P, KB, P], bf16, tag="ps_t")
                for k in range(KB):
                    nc.tensor.transpose(ps_xt[:, k, :], xn[:, ts(k, P)], ident)
                xT = xt_pool.tile([P, KB, P], bf16)
                (copy_eng or nc.vector.tensor_copy)(out=xT, in_=ps_xt)
                return (xT, sq)

            def transpose_tile_dma(fr, eng):
                # steady state: the DMA xbar does the [tok,d]->[d,tok]
                # transpose (out[di,k,t] = in[t, k*128+di]), freeing the PE
                # of ~14us of transposes and the DVE of the PSUM copies
                xn, sq = fr
                xT = xt_pool.tile([P, KB, P], bf16)
                eng.dma_start_transpose(xT[:, :, :], xn[:, :])
                return (xT, sq)

            fronts = {0: [transpose_tile(fr) for fr in frs.pop(0)]}

            # W prep tail: the last-landing chunks (scratch-abs on ACT runs
            # concurrently with their sgnh on DVE)
            emit_sgnh(2)
            emit_abs(2)
            emit_sgnh(7)
            emit_abs(7)
            asum1 = singles.tile([P, 1], fp32)
            nc.vector.tensor_reduce(
                out=asum1, in_=asum, axis=mybir.AxisListType.X, op=OP.add
            )

            for i in range(N_DUMMY2):
                nc.tensor.transpose(ps_dummy[:, i % KB, :], ident, ident)

            # cross-partition total broadcast to all partitions in ONE matmul
            ps_tot = ps_y_pool.tile([P, U], fp32, tag="ps_y", name="ps_tot")
            nc.tensor.matmul(
                ps_tot[:, 0:1], lhsT=ones_f32, rhs=asum1, start=True, stop=True
            )
            t128 = singles.tile([P, 1], fp32)
            nc.vector.tensor_copy(out=t128, in_=ps_tot[:, 0:1])
            # c = (beta+EPS)/2 ;  output scale 2*beta (wq holds 0.5*w_q)
            c128 = singles.tile([P, 1], fp32)
            nc.vector.tensor_scalar(
                out=c128, in0=t128, scalar1=0.5 / (D * U), scalar2=0.5 * EPS,
                op0=OP.mult, op1=OP.add,
            )
            bh128 = singles.tile([P, 1], fp32)
            nc.vector.tensor_scalar(
                out=bh128, in0=t128, scalar1=2.0 / (D * U), scalar2=None,
                op0=OP.mult,
            )

            # ---- ternarize: fused (|W| is_gt c) * sgnh per k-group ----
            wq = singles.tile([P, KB, U], bf16)  # holds 0.5*w_q (*gamma)

            def emit_tern(k0, klen):
                if k0 in SCRATCH_ABS:
                    i0 = SCRATCH_ABS.index(k0)
                    assert all(
                        SCRATCH_ABS.index(k0 + q) == i0 + q for q in range(klen)
                    )
                    src = scratch[:, i0 : i0 + klen, :]
                else:
                    # non-scratch chunks hold |W| in place in w_sb
                    assert all(
                        k0 + q not in SCRATCH_ABS for q in range(klen)
                    )
                    src = w_sb[:, k0 : k0 + klen, :]
                nc.vector.scalar_tensor_tensor(
                    out=wq[:, k0 : k0 + klen, :], in0=src, scalar=c128,
                    in1=sgnh[:, k0 : k0 + klen, :], op0=OP.is_gt, op1=OP.mult,
                )

            beff128 = None

            # ---- back side ----
            def esc_for(sq):
                # esc = rsqrt(var+eps) * 2*beta, per token (tiny DVE chain)
                esc = stats_pool.tile([P, 1], fp32, tag="esc")
                nc.vector.reciprocal(esc, sq)
                nc.vector.tensor_scalar(
                    out=esc, in0=esc, scalar1=bh128, scalar2=None, op0=OP.mult
                )
                return esc

            def epilogue(y_sb, i, j, ps_y, esc, h=None):
                sl = slice(None) if h is None else ts(h, 512)
                nc.scalar.mul(out=y_sb[:, i, sl], in_=ps_y[:, sl], mul=esc)
                if apply_beta:
                    nc.vector.tensor_tensor(
                        y_sb[:, i, sl], y_sb[:, i, sl], beff128[:, sl], OP.add
                    )

            def back_tile(xt_sq, y_sb, i, j):
                xT, sq = xt_sq
                last = j == NJ - 1
                ps_y = ps_y_pool.tile([P, U], fp32, tag="ps_y")
                esc = esc_for(sq)
                if last:
                    # h-major: each half's epilogue + 256 KiB drain starts as
                    # soon as that half's accumulation closes (short tail)
                    for h in range(2):
                        for k in range(KB):
                            nc.tensor.matmul(
                                ps_y[:, ts(h, 512)],
                                lhsT=xT[:, k, :],
                                rhs=wq[:, k, ts(h, 512)],
                                start=(k == 0),
                                stop=(k == KB - 1),
                            )
                        epilogue(y_sb, i, j, ps_y, esc, h)
                        eng = nc.scalar if h == 0 else nc.sync
                        eng.dma_start(
                            out=y_view[:, j * SUPER + i, ts(h, 512)],
                            in_=y_sb[:, i, ts(h, 512)],
                        )
                else:
                    for k in range(KB):
                        for h in range(2):
                            nc.tensor.matmul(
                                ps_y[:, ts(h, 512)],
                                lhsT=xT[:, k, :],
                                rhs=wq[:, k, ts(h, 512)],
                                start=(k == 0),
                                stop=(k == KB - 1),
                            )
                    epilogue(y_sb, i, j, ps_y, esc)

            def drain_y(j, y_sb):
                if j != NJ - 1:
                    nc.scalar.dma_start(
                        out=y_view[:, j * SUPER : (j + 1) * SUPER, :], in_=y_sb
                    )

            if not (apply_gamma or apply_beta):
                # ---- fast path: ternarize with super-1 stats interleaved;
                # the first super's two tiles then matmul k-interleaved so
                # consumption matches wq production with no PE stall.
                emit_tern(0, 2)
                emit_tern(2, 1)
                emit_tern(3, 2)
                fr1A = front_stats(x_supers[1], 0)
                emit_tern(5, 2)
                emit_tern(7, 1)
                fr1B = front_stats(x_supers[1], 1)
                frs[1] = [fr1A, fr1B]

                y_sb0 = y_pool.tile([P, SUPER, U], fp32)
                (xtA, sqA), (xtB, sqB) = fronts.pop(0)
                psA = ps_y_pool.tile([P, U], fp32, tag="ps_y")
                psB = ps_y_pool.tile([P, U], fp32, tag="ps_y")
                escA, escB = esc_for(sqA), esc_for(sqB)
                for k in range(KB):
                    for ps, xt in ((psA, xtA), (psB, xtB)):
                        for h in range(2):
                            nc.tensor.matmul(
                                ps[:, ts(h, 512)],
                                lhsT=xt[:, k, :],
                                rhs=wq[:, k, ts(h, 512)],
                                start=(k == 0),
                                stop=(k == KB - 1),
                            )
                epilogue(y_sb0, 0, 0, psA, escA)
                epilogue(y_sb0, 1, 0, psB, escB)
                drain_y(0, y_sb0)
            else:
                for q, ln in ((0, 2), (2, 1), (3, 2), (5, 2), (7, 1)):
                    emit_tern(q, ln)
                if apply_beta:
                    ps_beff = ps_y_pool.tile(
                        [P, U], fp32, tag="ps_y", name="ps_bf"
                    )
                    for k in range(KB):
                        for h in range(2):
                            nc.tensor.matmul(
                                ps_beff[0:1, ts(h, 512)],
                                lhsT=lb_sb[:, k : k + 1],
                                rhs=wq[:, k, ts(h, 512)],
                                start=(k == 0),
                                stop=(k == KB - 1),
                            )
                    beff = singles.tile([1, U], fp32)
                    nc.vector.tensor_scalar(
                        out=beff, in0=ps_beff[0:1, :], scalar1=bh128[0:1, 0:1],
                        scalar2=None, op0=OP.mult,
                    )
                    ps_b2 = ps_y_pool.tile([P, U], fp32, tag="ps_y")
                    ones_row = singles.tile([1, P], fp32)
                    nc.vector.memset(ones_row, 1.0)
                    for h in range(2):
                        nc.tensor.matmul(
                            ps_b2[:, ts(h, 512)], lhsT=ones_row,
                            rhs=beff[:, ts(h, 512)], start=True, stop=True,
                        )
                    beff128 = singles.tile([P, U], fp32)
                    nc.vector.tensor_copy(out=beff128, in_=ps_b2)
                    if apply_gamma:
                        for k in range(KB):
                            nc.vector.tensor_scalar(
                                out=wq[:, k, :], in0=wq[:, k, :],
                                scalar1=g_sb[:, k : k + 1], scalar2=None,
                                op0=OP.mult,
                            )
                frs[0] = [front_stats(x_supers[0], i) for i in range(SUPER)]
                frs[1] = [front_stats(x_supers[1], i) for i in range(SUPER)]
                f0 = [transpose_tile(fr) for fr in frs.pop(0)]
                y_sb0 = y_pool.tile([P, SUPER, U], fp32)
                for i in range(SUPER):
                    back_tile(f0[i], y_sb0, i, 0)
                drain_y(0, y_sb0)

            # super-1 transposes + copies; then x2 stats
            fronts[1] = [transpose_tile(fr) for fr in frs.pop(1)]
            x_supers[2] = issue_x(2, nc.sync)
            frs[2] = [front_stats(x_supers[2], i) for i in range(SUPER)]

            # ---- steady loop, per-tile interleave: M(j)i then T(j+1)i ----
            for j in range(1, NJ):
                y_sb = y_pool.tile([P, SUPER, U], fp32)
                xts = fronts.pop(j)
                nxt = [] if j + 1 < NJ else None
                for i in range(SUPER):
                    back_tile(xts[i], y_sb, i, j)
                    if nxt is not None:
                        nxt.append(transpose_tile(frs[j + 1][i]))
                if nxt is not None:
                    del frs[j + 1]
                    fronts[j + 1] = nxt
                drain_y(j, y_sb)
                if j + 2 < NJ:
                    x_supers[j + 2] = issue_x(j + 2, nc.sync)
                    frs[j + 2] = [
                        front_stats(x_supers[j + 2], i) for i in range(SUPER)
                    ]

    nc.compile()
    return nc


def _get_nc(apply_gamma: bool, apply_beta: bool):
    key = (apply_gamma, apply_beta)
    if key not in _NC_CACHE:
        _NC_CACHE[key] = _build(apply_gamma, apply_beta)
    return _NC_CACHE[key]


def _make_in_maps(x, w, g, lb, apply_gamma, apply_beta):
    xf = np.ascontiguousarray(x.reshape(B * S, D))
    in_maps = []
    for c in range(N_CORES):
        m = {
            "x": np.ascontiguousarray(xf[c * TOK : (c + 1) * TOK]),
            "weight": w,
        }
        if apply_gamma:
            m["ln_gamma"] = g
        if apply_beta:
            m["ln_beta"] = lb
        in_maps.append(m)
    return in_maps


def run(inputs, trace=False, tmpdir=None):
    """Shard, run on 8 cores, gather. Returns (y, BassKernelResults)."""
    from concourse.bass_utils import run_bass_kernel_spmd

    x = np.asarray(inputs["x"], dtype=np.float32)
    w = np.ascontiguousarray(np.asarray(inputs["weight"], dtype=np.float32))
    g = np.ascontiguousarray(np.asarray(inputs["ln_gamma"], dtype=np.float32))
    lb = np.ascontiguousarray(np.asarray(inputs["ln_beta"], dtype=np.float32))
    apply_gamma = not bool(np.all(g == 1.0))
    apply_beta = not bool(np.all(lb == 0.0))

    nc = _get_nc(apply_gamma, apply_beta)
    in_maps = _make_in_maps(x, w, g, lb, apply_gamma, apply_beta)
    res = run_bass_kernel_spmd(
        nc, in_maps, core_ids=list(range(N_CORES)), trace=trace, tmpdir=tmpdir
    )
    y = np.concatenate([r["y"] for r in res.results], axis=0)
    return y.reshape(B, S, U).astype(np.float32), res


def kernel(**inputs) -> np.ndarray:
    y, _ = run(inputs, trace=False)
    return y


# revision 54
# speedup vs baseline: 1.0344x; 1.0011x over previous
"""Trainium2 Bass kernel for nn_BitLinear (LayerNorm -> 1.58-bit BitLinear).

Math notes
----------
Reference computes, per the module:
    xn    = LN(x) * ln_gamma + ln_beta            (eps = 1e-3)
    beta  = mean(|W|);  w_q = clip(round(W / (beta + 1e-5)), -1, 1)
    gamma = max(|xn|)   (global absmax)
    xq    = clip(xn * 128 / gamma, -128 + 1e-5, 128 - 1e-5)
    y     = (xq @ w_q) * (gamma * beta / 128)

The gamma factor cancels exactly: (xn*128/gamma) @ w_q * (gamma*beta/128)
== (xn @ w_q) * beta.  The clip only affects elements within relative
7.8e-8 of the global absmax, changing them by ~1e-7 relative -- far below
f32 matmul roundoff.  So the kernel computes y = (LN(x) @ w_q) * beta,
which is fully data-parallel over tokens (no collectives needed).

w_q is ternary: w_q = sign(W) * 1[|W| > c] with c = 0.5*(beta+1e-5).
The kernel stores wq' = 0.5*w_q via ONE fused DVE op per k-group:
    wq = (|W| is_gt c) * sgnh,   sgnh = (W>=0)-0.5 in {-.5,+.5}
(scalar_tensor_tensor; the 2x is folded into the output scale 2*beta).
sgnh and |W| (f32, row-sums accumulated for beta) are computed while W
streams in.  All compares are f32: a bf16 compare would misclassify
~300 weights near the threshold (~2e-2 output error, at the budget).

LN normalization scale is folded into the epilogue: xn = (x - mu) in
bf16 (scale-invariant relative precision; the matmul is linear), and
esc[t] = rsqrt(var+eps)[t] * 2*beta scales each output row (ACT
scalar.mul with a [P,1] operand).

Sharding: data-parallel over the 32768 tokens, 4096 per core; weight
replicated (each core redundantly computes beta/w_q from the full W --
cheaper than a collective).

Schedule (measured: DMA starts ~8-10us after kernel start due to the
fixed engine preamble; the two HWDGE rings sustain ~410 GB/s combined
but share it unevenly; DVE f32 passes are ~0.7us, fused stt ~1.2us/k;
GPSIMD tensor ops are 2.5us AND stall concurrent DVE ops via SBUF
contention -- never used):
  * Ring q1 (sync/SP):    x0, W3a, W0, W1, W2, x1, x2, ...
    Ring q10 (scalar/ACT): W3b, W4..W7, y0, y1, ...
    (W3 split across rings balances their drain at ~23.5us; x supers
    queue behind W by ring FIFO so they cannot steal prologue
    bandwidth.)
  * W prep per chunk, in landing order: DVE extracts sgnh, then |W| in
    place via one fused pass for early q1 chunks, ACT Abs+accum for q10
    chunks; the LAST chunk on each ring (2 and 7) writes |W| to scratch
    instead (not in place) so its sgnh runs concurrently and the
    beta -> c chain closes ~1us after the last W byte.
  * The PE would idle >12us waiting for W; idle >3.4us drops its clock
    to 1.2 GHz (HAM gate).  Dummy identity transposes keep it busy+warm
    until real work arrives.
  * Ternarize emits as k-grouped fused ops with super-1 stats
    interleaved; the first super's matmuls interleave its two tiles
    k-by-k so consumption (~1.7us/k) matches wq production (~1.2us/k)
    with no PE stall.
  * Steady loop per tile: M(j)i then T(j+1)i -- the xT PSUM->SBUF copy
    for super j+1 runs on DVE during back(j)'s matmuls, so the PE never
    waits on copies.
  * Final super runs h-major with per-half drains on both rings, so the
    post-matmul tail is one 256 KiB transfer deep per ring.
"""

import numpy as np

B, S, D, U = 4, 8192, 1024, 1024
N_CORES = 8
TOK = (B * S) // N_CORES  # 4096 tokens per core
P = 128
KB = D // P               # 8 contraction blocks
NTILES = TOK // P         # 32 token tiles per core
SUPER = 2                 # token tiles per DMA transfer (1 MiB chunks)
NJ = NTILES // SUPER      # 16 super-tiles
N_DUMMY1 = 185            # PE warmup transposes before T0
N_DUMMY2 = 140            # ... between T0 and the ones-matmul
LN_EPS = 1e-3
EPS = 1e-5

_NC_CACHE = {}


def _build(apply_gamma: bool, apply_beta: bool):
    """Build the single-core Bass program (SPMD: same NEFF on all 8 cores)."""
    import concourse.bacc as bacc
    import concourse.mybir as mybir
    import concourse.tile as tile
    from concourse.bass import ts
    from concourse.masks import make_identity

    fp32 = mybir.dt.float32
    bf16 = mybir.dt.bfloat16
    AF = mybir.ActivationFunctionType
    OP = mybir.AluOpType

    nc = bacc.Bacc()
    x_h = nc.dram_tensor("x", [TOK, D], fp32, kind="ExternalInput")
    w_h = nc.dram_tensor("weight", [D, U], fp32, kind="ExternalInput")
    g_h = (
        nc.dram_tensor("ln_gamma", [D], fp32, kind="ExternalInput")
        if apply_gamma
        else None
    )
    lb_h = (
        nc.dram_tensor("ln_beta", [D], fp32, kind="ExternalInput")
        if apply_beta
        else None
    )
    y_h = nc.dram_tensor("y", [TOK, U], fp32, kind="ExternalOutput")

    DVE_ABS = (0, 1)      # chunks whose fused |W|+accum rides DVE
    SCRATCH_ABS = (2, 7)  # last chunk per ring: ACT abs to scratch

    with tile.TileContext(nc) as tc:
        with (
            tc.tile_pool(name="singles", bufs=1) as singles,
            tc.tile_pool(name="xin", bufs=4) as xin_pool,
            tc.tile_pool(name="xn", bufs=6) as xn_pool,
            tc.tile_pool(name="xt", bufs=8) as xt_pool,
            tc.tile_pool(name="yout", bufs=3) as y_pool,
            tc.tile_pool(name="stats", bufs=6) as stats_pool,
            tc.tile_pool(name="ps_t", bufs=4, space="PSUM") as ps_t_pool,
            tc.tile_pool(name="ps_y", bufs=2, space="PSUM") as ps_y_pool,
        ):
            # ---- constants ----
            ident = singles.tile([P, P], bf16)
            make_identity(nc, ident)
            eps_t = singles.tile([P, 1], fp32)
            nc.vector.memset(eps_t, LN_EPS)
            ones_f32 = singles.tile([P, P], fp32)
            nc.vector.memset(ones_f32, 1.0)

            # ---- DMA issue order defines ring FIFO order ----
            w_view = w_h[:, :].rearrange("(ko ki) u -> ki ko u", ki=P)
            x_view = x_h[:, :].rearrange("(o p) d -> p o d", p=P)
            y_view = y_h[:, :].rearrange("(o p) u -> p o u", p=P)

            def issue_x(j, eng):
                x_sb = xin_pool.tile([P, SUPER, D], fp32, name="x_sb")
                eng.dma_start(
                    out=x_sb, in_=x_view[:, j * SUPER : (j + 1) * SUPER, :]
                )
                return x_sb

            w_sb = singles.tile([P, KB, U], fp32)
            x_supers = {0: issue_x(0, nc.sync)}
            # q1: x0, W3a, W0, W1, W2, x1 ...   q10: W3b, W4..W7, y ...
            nc.sync.dma_start(out=w_sb[:, 3, 0:512], in_=w_view[:, 3, 0:512])
            nc.scalar.dma_start(
                out=w_sb[:, 3, 512:1024], in_=w_view[:, 3, 512:1024]
            )
            for k in (0, 1, 2):
                nc.sync.dma_start(out=w_sb[:, k, :], in_=w_view[:, k, :])
            for k in (4, 5, 6, 7):
                nc.scalar.dma_start(out=w_sb[:, k, :], in_=w_view[:, k, :])
            x_supers[1] = issue_x(1, nc.sync)

            if apply_gamma:
                g_sb = singles.tile([P, KB], fp32)
                nc.scalar.dma_start(
                    out=g_sb, in_=g_h[:].rearrange("(ko ki) -> ki ko", ki=P)
                )
            if apply_beta:
                lb_f32 = singles.tile([P, KB], fp32)
                nc.scalar.dma_start(
                    out=lb_f32, in_=lb_h[:].rearrange("(ko ki) -> ki ko", ki=P)
                )
                lb_sb = singles.tile([P, KB], bf16)
                nc.vector.tensor_copy(out=lb_sb, in_=lb_f32)

            # ---- W prep: sgnh (sign), then |W| + row-sums for beta ----
            sgnh = singles.tile([P, KB, U], bf16)
            asum = singles.tile([P, KB], fp32)
            scratch = singles.tile([P, 2, U], fp32)  # |W| for chunks 2 and 7
            abs_src = {}  # k -> AP holding |W| for the ternarize compare

            def emit_sgnh(k):
                # (W>=0)-0.5 in {-.5,+.5}, exact in bf16
                nc.vector.tensor_scalar(
                    out=sgnh[:, k, :], in0=w_sb[:, k, :], scalar1=0.0,
                    scalar2=0.5, op0=OP.is_ge, op1=OP.subtract,
                )
                if apply_gamma and not apply_beta:
                    # fold ln_gamma rows in (the beff path needs raw w_q, so
                    # the combined variant applies gamma later instead)
                    nc.vector.tensor_scalar(
                        out=sgnh[:, k, :], in0=sgnh[:, k, :],
                        scalar1=g_sb[:, k : k + 1], scalar2=None, op0=OP.mult,
                    )

            def emit_abs(k):
                if k in SCRATCH_ABS:
                    # NOT in place: sgnh (DVE) runs concurrently with this
                    # ACT pass -- critical for the last chunk on each ring
                    i = SCRATCH_ABS.index(k)
                    nc.scalar.activation(
                        out=scratch[:, i, :], in_=w_sb[:, k, :], func=AF.Abs,
                        accum_out=asum[:, k : k + 1],
                    )
                    abs_src[k] = scratch[:, i : i + 1, :]
                elif k in DVE_ABS:
                    # |W| = (2W) * sgnh exactly, row-sum accumulated: one
                    # fused DVE pass, in place
                    nc.vector.scalar_tensor_tensor(
                        out=w_sb[:, k, :], in0=w_sb[:, k, :], scalar=2.0,
                        in1=sgnh[:, k, :], op0=OP.mult, op1=OP.mult,
                        accum_out=asum[:, k : k + 1],
                    )
                    abs_src[k] = w_sb[:, k : k + 1, :]
                else:
                    nc.scalar.activation(
                        out=w_sb[:, k, :], in_=w_sb[:, k, :], func=AF.Abs,
                        accum_out=asum[:, k : k + 1],
                    )
                    abs_src[k] = w_sb[:, k : k + 1, :]

            # ---- LN stats on DVE; the normalize pass rides ACT ----
            def front_stats(x_sb, i):
                xt_ = x_sb[:, i, :]
                st = stats_pool.tile([P, 2, 6], fp32, tag="st")
                xr = xt_.rearrange("p (n f) -> p n f", f=512)
                nc.vector.bn_stats(out=st[:, 0, :], in_=xr[:, 0, :])
                nc.vector.bn_stats(out=st[:, 1, :], in_=xr[:, 1, :])
                mv = stats_pool.tile([P, 2], fp32, tag="mv")
                nc.vector.bn_aggr(out=mv, in_=st)
                nb = stats_pool.tile([P, 1], fp32, tag="nb")
                nc.vector.tensor_scalar(
                    out=nb, in0=mv[:, 0:1], scalar1=-1.0, scalar2=None,
                    op0=OP.mult,
                )
                # xn = x - mu (bf16); rsqrt scale folds into the epilogue
                xn = xn_pool.tile([P, D], bf16)
                nc.scalar.activation(
                    out=xn, in_=xt_, func=AF.Identity, bias=nb, scale=1.0
                )
                # sq = sqrt(var + eps) (tiny, ACT)
                sq = stats_pool.tile([P, 1], fp32, tag="sq")
                nc.scalar.activation(
                    out=sq, in_=mv[:, 1:2], func=AF.Sqrt, bias=eps_t, scale=1.0
                )
                return xn, sq

            # W prep in expected landing order (q10: W3b@12, W4@15, W5@19,
            # W6@22, W7@23.5; q1: x0@14, W3a@16, W0@19, W1@21, W2@23.5).
            # DVE chunks fuse sgnh-mult+accum in one in-place pass; the
            # last chunk per ring writes |W| to scratch so its sgnh (DVE)
            # runs concurrently with the ACT abs.
            emit_sgnh(4)
            emit_abs(4)
            frs = {0: [front_stats(x_supers[0], i) for i in range(SUPER)]}
            emit_sgnh(3)
            emit_abs(3)
            emit_sgnh(5)
            emit_abs(5)
            emit_sgnh(0)
            emit_abs(0)
            emit_sgnh(1)
            emit_abs(1)
            emit_sgnh(6)
            emit_abs(6)

            # ---- PE warmup dummies (keep the HAM clock at 2.4 GHz) ----
            ps_dummy = ps_t_pool.tile([P, KB, P], bf16, tag="ps_t", name="ps_d")
            for i in range(N_DUMMY1):
                nc.tensor.transpose(ps_dummy[:, i % KB, :], ident, ident)

            # ---- transposes + copies ----
            def transpose_tile(fr, copy_eng=None):
                xn, sq = fr
                ps_xt = ps_t_pool.tile([P, KB, P], bf16, tag="ps_t")
                for k in range(KB):
                    nc.tensor.transpose(ps_xt[:, k, :], xn[:, ts(k, P)], ident)
                xT = xt_pool.tile([P, KB, P], bf16)
                (copy_eng or nc.vector.tensor_copy)(out=xT, in_=ps_xt)
                return (xT, sq)

            # (note: dma_start_transpose SBUF->SBUF was tried for these and
            # produces wrong output on hardware -- PE transposes stay)
            fronts = {0: [transpose_tile(fr) for fr in frs.pop(0)]}

            # W prep tail: the last-landing chunks (scratch-abs on ACT runs
            # concurrently with their sgnh on DVE)
            emit_sgnh(2)
            emit_abs(2)
            emit_sgnh(7)
            emit_abs(7)
            asum1 = singles.tile([P, 1], fp32)
            nc.vector.tensor_reduce(
                out=asum1, in_=asum, axis=mybir.AxisListType.X, op=OP.add
            )

            for i in range(N_DUMMY2):
                nc.tensor.transpose(ps_dummy[:, i % KB, :], ident, ident)

            # cross-partition total broadcast to all partitions in ONE matmul
            ps_tot = ps_y_pool.tile([P, U], fp32, tag="ps_y", name="ps_tot")
            nc.tensor.matmul(
                ps_tot[:, 0:1], lhsT=ones_f32, rhs=asum1, start=True, stop=True
            )
            t128 = singles.tile([P, 1], fp32)
            nc.vector.tensor_copy(out=t128, in_=ps_tot[:, 0:1])
            # c = (beta+EPS)/2 ;  output scale 2*beta (wq holds 0.5*w_q)
            c128 = singles.tile([P, 1], fp32)
            nc.vector.tensor_scalar(
                out=c128, in0=t128, scalar1=0.5 / (D * U), scalar2=0.5 * EPS,
                op0=OP.mult, op1=OP.add,
            )
            bh128 = singles.tile([P, 1], fp32)
            nc.vector.tensor_scalar(
                out=bh128, in0=t128, scalar1=2.0 / (D * U), scalar2=None,
                op0=OP.mult,
            )

            # ---- ternarize: fused (|W| is_gt c) * sgnh per k-group ----
            wq = singles.tile([P, KB, U], bf16)  # holds 0.5*w_q (*gamma)

            def emit_tern(k0, klen):
                if k0 in SCRATCH_ABS:
                    i0 = SCRATCH_ABS.index(k0)
                    assert all(
                        SCRATCH_ABS.index(k0 + q) == i0 + q for q in range(klen)
                    )
                    src = scratch[:, i0 : i0 + klen, :]
                else:
                    # non-scratch chunks hold |W| in place in w_sb
                    assert all(
                        k0 + q not in SCRATCH_ABS for q in range(klen)
                    )
                    src = w_sb[:, k0 : k0 + klen, :]
                nc.vector.scalar_tensor_tensor(
                    out=wq[:, k0 : k0 + klen, :], in0=src, scalar=c128,
                    in1=sgnh[:, k0 : k0 + klen, :], op0=OP.is_gt, op1=OP.mult,
                )

            beff128 = None

            # ---- back side ----
            def esc_for(sq):
                # esc = rsqrt(var+eps) * 2*beta, per token (tiny DVE chain)
                esc = stats_pool.tile([P, 1], fp32, tag="esc")
                nc.vector.reciprocal(esc, sq)
                nc.vector.tensor_scalar(
                    out=esc, in0=esc, scalar1=bh128, scalar2=None, op0=OP.mult
                )
                return esc

            def epilogue(y_sb, i, j, ps_y, esc, h=None):
                sl = slice(None) if h is None else ts(h, 512)
                nc.scalar.mul(out=y_sb[:, i, sl], in_=ps_y[:, sl], mul=esc)
                if apply_beta:
                    nc.vector.tensor_tensor(
                        y_sb[:, i, sl], y_sb[:, i, sl], beff128[:, sl], OP.add
                    )

            def back_tile(xt_sq, y_sb, i, j):
                xT, sq = xt_sq
                last = j == NJ - 1
                ps_y = ps_y_pool.tile([P, U], fp32, tag="ps_y")
                esc = esc_for(sq)
                if last:
                    # h-major: each half's epilogue + 256 KiB drain starts as
                    # soon as that half's accumulation closes (short tail)
                    for h in range(2):
                        for k in range(KB):
                            nc.tensor.matmul(
                                ps_y[:, ts(h, 512)],
                                lhsT=xT[:, k, :],
                                rhs=wq[:, k, ts(h, 512)],
                                start=(k == 0),
                                stop=(k == KB - 1),
                            )
                        epilogue(y_sb, i, j, ps_y, esc, h)
                        eng = nc.scalar if h == 0 else nc.sync
                        eng.dma_start(
                            out=y_view[:, j * SUPER + i, ts(h, 512)],
                            in_=y_sb[:, i, ts(h, 512)],
                        )
                else:
                    for k in range(KB):
                        for h in range(2):
                            nc.tensor.matmul(
                                ps_y[:, ts(h, 512)],
                                lhsT=xT[:, k, :],
                                rhs=wq[:, k, ts(h, 512)],
                                start=(k == 0),
                                stop=(k == KB - 1),
                            )
                    epilogue(y_sb, i, j, ps_y, esc)

            def drain_y(j, y_sb):
                if j != NJ - 1:
                    nc.scalar.dma_start(
                        out=y_view[:, j * SUPER : (j + 1) * SUPER, :], in_=y_sb
                    )

            if not (apply_gamma or apply_beta):
                # ---- fast path: ternarize with super-1 stats interleaved;
                # the first super's two tiles then matmul k-interleaved so
                # consumption matches wq production with no PE stall.
                emit_tern(0, 2)
                emit_tern(2, 1)
                emit_tern(3, 2)
                fr1A = front_stats(x_supers[1], 0)
                emit_tern(5, 2)
                emit_tern(7, 1)
                fr1B = front_stats(x_supers[1], 1)
                frs[1] = [fr1A, fr1B]

                y_sb0 = y_pool.tile([P, SUPER, U], fp32)
                (xtA, sqA), (xtB, sqB) = fronts.pop(0)
                psA = ps_y_pool.tile([P, U], fp32, tag="ps_y")
                psB = ps_y_pool.tile([P, U], fp32, tag="ps_y")
                escA, escB = esc_for(sqA), esc_for(sqB)
                for k in range(KB):
                    for ps, xt in ((psA, xtA), (psB, xtB)):
                        for h in range(2):
                            nc.tensor.matmul(
                                ps[:, ts(h, 512)],
                                lhsT=xt[:, k, :],
                                rhs=wq[:, k, ts(h, 512)],
                                start=(k == 0),
                                stop=(k == KB - 1),
                            )
                epilogue(y_sb0, 0, 0, psA, escA)
                epilogue(y_sb0, 1, 0, psB, escB)
                drain_y(0, y_sb0)
            else:
                for q, ln in ((0, 2), (2, 1), (3, 2), (5, 2), (7, 1)):
                    emit_tern(q, ln)
                if apply_beta:
                    ps_beff = ps_y_pool.tile(
                        [P, U], fp32, tag="ps_y", name="ps_bf"
                    )
                    for k in range(KB):
                        for h in range(2):
                            nc.tensor.matmul(
                                ps_beff[0:1, ts(h, 512)],
                                lhsT=lb_sb[:, k : k + 1],
                                rhs=wq[:, k, ts(h, 512)],
                                start=(k == 0),
                                stop=(k == KB - 1),
                            )
                    beff = singles.tile([1, U], fp32)
                    nc.vector.tensor_scalar(
                        out=beff, in0=ps_beff[0:1, :], scalar1=bh128[0:1, 0:1],
                        scalar2=None, op0=OP.mult,
                    )
                    ps_b2 = ps_y_pool.tile([P, U], fp32, tag="ps_y")
                    ones_row = singles.tile([1, P], fp32)
                    nc.vector.memset(ones_row, 1.0)
                    for h in range(2):
                        nc.tensor.matmul(
                            ps_b2[:, ts(h, 512)], lhsT=ones_row,
                            rhs=beff[:, ts(h, 512)], start=True, stop=True,
                        )
                    beff128 = singles.tile([P, U], fp32)
                    nc.vector.tensor_copy(out=beff128, in_=ps_b2)
                    if apply_gamma:
                        for k in range(KB):
                            nc.vector.tensor_scalar(
                                out=wq[:, k, :], in0=wq[:, k, :],
                                scalar1=g_sb[:, k : k + 1], scalar2=None,
                                op0=OP.mult,
                            )
                frs[0] = [front_stats(x_supers[0], i) for i in range(SUPER)]
                frs[1] = [front_stats(x_supers[1], i) for i in range(SUPER)]
                f0 = [transpose_tile(fr) for fr in frs.pop(0)]
                y_sb0 = y_pool.tile([P, SUPER, U], fp32)
                for i in range(SUPER):
                    back_tile(f0[i], y_sb0, i, 0)
                drain_y(0, y_sb0)

            # super-1 transposes + copies; then x2 stats
            fronts[1] = [transpose_tile(fr) for fr in frs.pop(1)]
            x_supers[2] = issue_x(2, nc.sync)
            frs[2] = [front_stats(x_supers[2], i) for i in range(SUPER)]

            # ---- steady loop, per-tile interleave: M(j)i then T(j+1)i ----
            for j in range(1, NJ):
                y_sb = y_pool.tile([P, SUPER, U], fp32)
                xts = fronts.pop(j)
                nxt = [] if j + 1 < NJ else None
                for i in range(SUPER):
                    back_tile(xts[i], y_sb, i, j)
                    if nxt is not None:
                        nxt.append(transpose_tile(frs[j + 1][i]))
                if nxt is not None:
                    del frs[j + 1]
                    fronts[j + 1] = nxt
                drain_y(j, y_sb)
                if j + 2 < NJ:
                    x_supers[j + 2] = issue_x(j + 2, nc.sync)
                    frs[j + 2] = [
                        front_stats(x_supers[j + 2], i) for i in range(SUPER)
                    ]

    nc.compile()
    return nc


def _get_nc(apply_gamma: bool, apply_beta: bool):
    key = (apply_gamma, apply_beta)
    if key not in _NC_CACHE:
        _NC_CACHE[key] = _build(apply_gamma, apply_beta)
    return _NC_CACHE[key]


def _make_in_maps(x, w, g, lb, apply_gamma, apply_beta):
    xf = np.ascontiguousarray(x.reshape(B * S, D))
    in_maps = []
    for c in range(N_CORES):
        m = {
            "x": np.ascontiguousarray(xf[c * TOK : (c + 1) * TOK]),
            "weight": w,
        }
        if apply_gamma:
            m["ln_gamma"] = g
        if apply_beta:
            m["ln_beta"] = lb
        in_maps.append(m)
    return in_maps


def run(inputs, trace=False, tmpdir=None):
    """Shard, run on 8 cores, gather. Returns (y, BassKernelResults)."""
    from concourse.bass_utils import run_bass_kernel_spmd

    x = np.asarray(inputs["x"], dtype=np.float32)
    w = np.ascontiguousarray(np.asarray(inputs["weight"], dtype=np.float32))
    g = np.ascontiguousarray(np.asarray(inputs["ln_gamma"], dtype=np.float32))
    lb = np.ascontiguousarray(np.asarray(inputs["ln_beta"], dtype=np.float32))
    apply_gamma = not bool(np.all(g == 1.0))
    apply_beta = not bool(np.all(lb == 0.0))

    nc = _get_nc(apply_gamma, apply_beta)
    in_maps = _make_in_maps(x, w, g, lb, apply_gamma, apply_beta)
    res = run_bass_kernel_spmd(
        nc, in_maps, core_ids=list(range(N_CORES)), trace=trace, tmpdir=tmpdir
    )
    y = np.concatenate([r["y"] for r in res.results], axis=0)
    return y.reshape(B, S, U).astype(np.float32), res


def kernel(**inputs) -> np.ndarray:
    y, _ = run(inputs, trace=False)
    return y
